# revision 1
# baseline (speedup 1.0000x reference)
"""Trainium2 Bass kernel for nn_DiagonalSSM (4-layer diagonal-SSM LM).

Sharding (8 cores):
  - Trunk: token-sharded. Core k handles batch k//4, tokens
    [(k%4)*512, (k%4+1)*512). The SSM scan runs as chunked scaled-cumsums on
    the PE; cross-segment carries use one tiny AllGather (2 groups of 4) per
    layer, launched early (G computed directly from Bu) so it overlaps the
    scan itself.
  - Head: vocab-sharded. After the final LN, activations are AllGathered
    (all 8 cores, split into two halves for overlap) and each core computes
    logits for its 4000-vocab slice over all 4096 tokens.

Layout: activations feature-major ([d, t]); residual stream kept in f32r.
All large matmuls run in float32r (full PE rate, ~13-bit mantissa); the
scan cumsum runs in fp32.
"""

import numpy as np

L, D, S, V = 4, 512, 256, 32000
DFF = 1368
B, T = 2, 2048
NCORES = 8
NSEG = 4
TSEG = 512
VSH = V // NCORES  # 4000
NVC = 8
VC = VSH // NVC    # 500
EPS = 1e-5
CH = 128
NCH = TSEG // CH   # 4
ND = D // 128      # 4
NS = S // 128      # 2
NFT = (DFF + 127) // 128  # 11
FTS = [128] * (DFF // 128) + ([DFF % 128] if DFF % 128 else [])
NLC = 6 * L + 2    # packed LN-param columns

_NC_CACHE = {}


def _build_nc(debug=False):
    import concourse.bass as bass
    import concourse.tile as tile
    from concourse import bacc, mybir
    from concourse.masks import make_identity

    f32 = mybir.dt.float32
    f32r = mybir.dt.float32r
    i32 = mybir.dt.int32
    AF = mybir.ActivationFunctionType
    OP = mybir.AluOpType

    nc = bacc.Bacc("TRN2", target_bir_lowering=False, debug=False,
                   num_devices=NCORES)

    # ---------------- DRAM I/O ----------------
    d_x = nc.dram_tensor("x_seg", [TSEG, 1], i32, kind="ExternalInput")
    d_emb = nc.dram_tensor("emb", [V, D], f32, kind="ExternalInput")
    d_pos = nc.dram_tensor("pos_seg", [TSEG, D], f32, kind="ExternalInput")
    d_BwT = nc.dram_tensor("BwT", [L, D, S], f32r, kind="ExternalInput")
    d_CwT = nc.dram_tensor("CwT", [L, S, D], f32r, kind="ExternalInput")
    d_w1T = nc.dram_tensor("w1T", [L, D, DFF], f32r, kind="ExternalInput")
    d_w2T = nc.dram_tensor("w2T", [L, D, DFF], f32r, kind="ExternalInput")
    d_w3T = nc.dram_tensor("w3T", [L, DFF, D], f32r, kind="ExternalInput")
    d_hWT = nc.dram_tensor("headWT", [D, VSH], f32r, kind="ExternalInput")
    d_hbb = nc.dram_tensor("headb_bc", [128, VSH], f32, kind="ExternalInput")
    d_lnc = nc.dram_tensor("lncols", [D, NLC], f32, kind="ExternalInput")
    d_laminv = nc.dram_tensor("laminv", [L, CH, S], f32, kind="ExternalInput")
    d_lamp = nc.dram_tensor("lamp", [L, CH, S], f32, kind="ExternalInput")
    d_chc = nc.dram_tensor("chc", [L, CH, S], f32, kind="ExternalInput")
    d_lamw = nc.dram_tensor("lamw", [L, NCH, S], f32, kind="ExternalInput")
    d_chc2T = nc.dram_tensor("chc2T", [L, S, TSEG], f32, kind="ExternalInput")
    d_lamc = nc.dram_tensor("lamc", [L, NCH, NCH, S], f32, kind="ExternalInput")
    d_segc = nc.dram_tensor("segcoef", [L, NCH, S], f32, kind="ExternalInput")
    d_U = nc.dram_tensor("Utri", [CH, CH], f32, kind="ExternalInput")
    d_ones4 = nc.dram_tensor("ones4", [NCH, 128], f32, kind="ExternalInput")
    d_ones4f = nc.dram_tensor("ones4f", [NCH, TSEG], f32, kind="ExternalInput")
    d_ones128r = nc.dram_tensor("ones128r", [128, 128], f32r, kind="ExternalInput")
    d_onesD = nc.dram_tensor("onesD", [128, 128], f32r, kind="ExternalInput")

    d_out = nc.dram_tensor("logits", [B * T, VSH], f32, kind="ExternalOutput")

    dbg = {}
    if debug:
        for nm, shp in (
            ("dbg_h0", [D, TSEG]), ("dbg_xn", [D, TSEG]),
            ("dbg_bu", [CH, NCH * S]), ("dbg_loc", [CH, NCH * S]),
            ("dbg_hst", [S, TSEG]), ("dbg_h1", [D, TSEG]),
            ("dbg_xnf", [D, TSEG]),
        ):
            dbg[nm] = nc.dram_tensor(nm, shp, f32, kind="ExternalOutput")

    with tile.TileContext(nc) as tc:
        with (
            tc.tile_pool(name="const", bufs=1) as cpool,
            tc.tile_pool(name="hm", bufs=1) as hm,
            tc.tile_pool(name="scr", bufs=2) as scr,
            tc.tile_pool(name="one", bufs=1) as one,
            tc.tile_pool(name="ps", bufs=1, space="PSUM") as ps,
            tc.tile_pool(name="ps4", bufs=4, space="PSUM") as ps4,
            tc.tile_pool(name="dram", bufs=1, space="DRAM") as dram,
        ):
            # ---------------- constants ----------------
            ident = cpool.tile([128, 128], f32, name="ident")
            make_identity(nc, ident[:, :])
            U_sb = cpool.tile([CH, CH], f32, name="U_sb")
            nc.sync.dma_start(out=U_sb[:, :], in_=d_U[:, :])
            ones4_sb = cpool.tile([NCH, 128], f32, name="ones4_sb")
            nc.sync.dma_start(out=ones4_sb[:, :], in_=d_ones4[:, :])
            ones4f_sb = cpool.tile([NCH, TSEG], f32, name="ones4f_sb")
            nc.sync.dma_start(out=ones4f_sb[:, :], in_=d_ones4f[:, :])
            ones128r_sb = cpool.tile([128, 128], f32r, name="ones128r_sb")
            nc.sync.dma_start(out=ones128r_sb[:, :], in_=d_ones128r[:, :])
            onesD_sb = cpool.tile([128, 128], f32r, name="onesD_sb")
            nc.sync.dma_start(out=onesD_sb[:, :], in_=d_onesD[:, :])
            eps_sb = cpool.tile([128, 1], f32, name="eps_sb")
            nc.vector.memset(eps_sb[:, :], EPS)
            lnc_sb = []
            for dd in range(ND):
                t = cpool.tile([128, NLC], f32, tag=f"lnc{dd}", name=f"lnc{dd}")
                nc.sync.dma_start(
                    out=t[:, :], in_=d_lnc[dd * 128:(dd + 1) * 128, :])
                lnc_sb.append(t)

            def lncol(key, ll, dd):
                base = {"n1w": 0, "n1b": L, "n2w": 2 * L, "n2b": 3 * L,
                        "Dpw": 4 * L, "Dpb": 5 * L + 2}
                if key == "noww":
                    c = 5 * L
                elif key == "nob":
                    c = 5 * L + 1
                else:
                    c = base[key] + ll
                return lnc_sb[dd][:, c:c + 1]

            # ---------------- h master (feature-major, f32r) --------------
            h = [hm.tile([128, TSEG], f32r, tag=f"h{dd}", name=f"h{dd}")
                 for dd in range(ND)]
            act_ctx = tc.tile_pool(name="act", bufs=1)
            act = act_ctx.__enter__()

            # ---------------- embedding ----------------
            for tt in range(NCH):
                idx_sb = scr.tile([128, 1], i32, tag="idx", name="idx")
                nc.sync.dma_start(
                    out=idx_sb[:, :], in_=d_x[tt * 128:(tt + 1) * 128, :])
                e_t = one.tile([128, D], f32, tag="e_t", name="e_t")
                nc.gpsimd.indirect_dma_start(
                    out=e_t[:, :], out_offset=None, in_=d_emb[:, :],
                    in_offset=bass.IndirectOffsetOnAxis(ap=idx_sb[:, :1], axis=0),
                )
                p_t = one.tile([128, D], f32, tag="p_t", name="p_t")
                nc.sync.dma_start(
                    out=p_t[:, :], in_=d_pos[tt * 128:(tt + 1) * 128, :])
                htm = one.tile([128, D], f32, tag="htm", name="htm")
                nc.vector.tensor_tensor(
                    out=htm[:, :], in0=e_t[:, :], in1=p_t[:, :], op=OP.add)
                for dd in range(ND):
                    trp = ps.tile([128, 128], f32,
                                  tag=("sm" if dd % 2 == 0 else "gps"), name="trp")
                    nc.tensor.transpose(
                        trp[:, :], htm[:, dd * 128:(dd + 1) * 128], ident[:, :])
                    nc.vector.tensor_copy(
                        out=h[dd][:, tt * 128:(tt + 1) * 128], in_=trp[:, :])

            def dump_fm(key, tiles):
                for dd in range(len(tiles)):
                    o = one.tile([128, TSEG], f32, tag="dbgcp", name="dbgcp")
                    nc.vector.tensor_copy(out=o[:, :], in_=tiles[dd][:, :])
                    nc.sync.dma_start(
                        out=dbg[key][dd * 128:(dd + 1) * 128, :], in_=o[:, :])

            if debug:
                dump_fm("dbg_h0", h)

            # ---------------- LN helper ----------------
            def ln_stats():
                # mean and E[x^2] accumulate in parallel (separate banks)
                mu = ps.tile([128, TSEG], f32, tag="sm", name="mu")
                ex2 = ps.tile([128, TSEG], f32, tag="gps", name="ex2")
                for dd in range(ND):
                    nc.tensor.matmul(mu[:, :], onesD_sb[:, :], h[dd][:, :],
                                     start=(dd == 0), stop=(dd == ND - 1))
                for dd in range(ND):
                    h2 = scr.tile([128, TSEG], f32r, tag="h2", name="h2")
                    nc.vector.tensor_tensor(
                        out=h2[:, :], in0=h[dd][:, :], in1=h[dd][:, :], op=OP.mult)
                    nc.tensor.matmul(ex2[:, :], onesD_sb[:, :], h2[:, :],
                                     start=(dd == 0), stop=(dd == ND - 1))
                mu_sb = one.tile([128, TSEG], f32, tag="mu_sb", name="mu_sb")
                nc.vector.tensor_copy(out=mu_sb[:, :], in_=mu[:, :])
                var = one.tile([128, TSEG], f32, tag="var_sb", name="var_sb")
                nc.vector.tensor_tensor(
                    out=var[:, :], in0=mu_sb[:, :], in1=mu[:, :], op=OP.mult)
                nc.vector.tensor_tensor(
                    out=var[:, :], in0=ex2[:, :], in1=var[:, :], op=OP.subtract)
                sd = one.tile([128, TSEG], f32, tag="sd", name="sd")
                nc.scalar.activation(out=sd[:, :], in_=var[:, :], func=AF.Sqrt,
                                     bias=eps_sb[:, :], scale=1.0)
                rstd = one.tile([128, TSEG], f32, tag="rstd", name="rstd")
                nc.vector.reciprocal(out=rstd[:, :], in_=sd[:, :])
                return mu_sb, sd, rstd

            def layer_norm(w_key, b_key, ll=None, out_tag="xn"):
                mu_sb, sd, rstd = ln_stats()
                hc = []
                for dd in range(ND):
                    c = act.tile([128, TSEG], f32r, tag=f"hc{dd}", name=f"hc{dd}")
                    eng = nc.vector if dd < 2 else nc.gpsimd
                    eng.tensor_tensor(
                        out=c[:, :], in0=h[dd][:, :], in1=mu_sb[:, :],
                        op=OP.subtract)
                    hc.append(c)
                xn = []
                for dd in range(ND):
                    eng = nc.vector if dd < 2 else nc.gpsimd
                    t1 = scr.tile([128, TSEG], f32, tag="lnt1", name="lnt1")
                    eng.tensor_tensor(
                        out=t1[:, :], in0=hc[dd][:, :], in1=rstd[:, :], op=OP.mult)
                    xo = act.tile([128, TSEG], f32r, tag=f"{out_tag}{dd}",
                                  name=f"{out_tag}{dd}")
                    eng.tensor_scalar(
                        out=xo[:, :], in0=t1[:, :],
                        scalar1=lncol(w_key, ll, dd), scalar2=lncol(b_key, ll, dd),
                        op0=OP.mult, op1=OP.add)
                    xn.append(xo)
                return xn

            # ---------------- layers ----------------
            with (
                tc.tile_pool(name="tabs", bufs=1) as tabs,
                tc.tile_pool(name="wbig", bufs=1) as wbig,
            ):
                for ll in range(L):
                    laminv_sb = tabs.tile([CH, S], f32, tag="laminv", name="laminv")
                    lamp_sb = tabs.tile([CH, S], f32, tag="lamp", name="lamp")
                    chc_sb = tabs.tile([CH, S], f32, tag="chc", name="chc")
                    lamw_sb = tabs.tile([1, NCH, S], f32, tag="lamw", name="lamw")
                    chc2T_sb = [tabs.tile([128, TSEG], f32, tag=f"chc2T{ss}",
                                          name=f"chc2T{ss}") for ss in range(NS)]
                    lamc_sb = tabs.tile([NCH, NCH, S], f32, tag="lamc", name="lamc")
                    segc_sb = tabs.tile([NCH, S], f32, tag="segc", name="segc")
                    nc.sync.dma_start(out=laminv_sb[:, :], in_=d_laminv[ll])
                    nc.sync.dma_start(out=lamp_sb[:, :], in_=d_lamp[ll])
                    nc.sync.dma_start(out=chc_sb[:, :], in_=d_chc[ll])
                    nc.sync.dma_start(out=lamw_sb[0:1, :, :], in_=d_lamw[ll])
                    for ss in range(NS):
                        nc.sync.dma_start(
                            out=chc2T_sb[ss][:, :],
                            in_=d_chc2T[ll, ss * 128:(ss + 1) * 128, :])
                    nc.sync.dma_start(out=lamc_sb[:, :, :], in_=d_lamc[ll])
                    nc.sync.dma_start(out=segc_sb[:, :], in_=d_segc[ll])
                    BwT_sb = []
                    for dd in range(ND):
                        t = tabs.tile([128, S], f32r, tag=f"BwT{dd}",
                                      name=f"BwT{dd}")
                        nc.sync.dma_start(
                            out=t[:, :], in_=d_BwT[ll, dd * 128:(dd + 1) * 128, :])
                        BwT_sb.append(t)
                    CwT_sb = []
                    for ss in range(NS):
                        t = tabs.tile([128, D], f32r, tag=f"CwT{ss}",
                                      name=f"CwT{ss}")
                        nc.sync.dma_start(
                            out=t[:, :], in_=d_CwT[ll, ss * 128:(ss + 1) * 128, :])
                        CwT_sb.append(t)
                    w1sb, w2sb = [], []
                    for dd in range(ND):
                        t = wbig.tile([128, DFF], f32r, tag=f"w1sb{dd}",
                                      name=f"w1sb{dd}")
                        nc.sync.dma_start(
                            out=t[:, :], in_=d_w1T[ll, dd * 128:(dd + 1) * 128, :])
                        w1sb.append(t)
                        t = wbig.tile([128, DFF], f32r, tag=f"w2sb{dd}",
                                      name=f"w2sb{dd}")
                        nc.sync.dma_start(
                            out=t[:, :], in_=d_w2T[ll, dd * 128:(dd + 1) * 128, :])
                        w2sb.append(t)

                    # per-layer folded Bw weights (off critical path)
                    BwTw = []
                    for dd in range(ND):
                        t = tabs.tile([128, S], f32r, tag=f"BwTw{dd}",
                                      name=f"BwTw{dd}")
                        nc.gpsimd.tensor_scalar_mul(
                            out=t[:, :], in0=BwT_sb[dd][:, :],
                            scalar1=lncol("n1w", ll, dd))
                        BwTw.append(t)

                    # ---- LN1 stats; P-matmuls start per-chunk on hc ----
                    mu_sb, sd, rstd = ln_stats()
                    # Bb[s] = sum_d n1b[d] BwT[d,s]; broadcast + pre-scale
                    n1br = []
                    for dd in range(ND):
                        t = scr.tile([128, 1], f32r, tag="n1br", name="n1br")
                        nc.vector.tensor_copy(out=t[:, :],
                                              in_=lncol("n1b", ll, dd))
                        n1br.append(t)
                    colb = ps.tile([CH, S], f32, tag="gps", name="colb")
                    for dd in range(ND):
                        nc.tensor.matmul(colb[0:1, :], n1br[dd][:, :],
                                         BwT_sb[dd][:, :],
                                         start=(dd == 0), stop=(dd == ND - 1))
                    Bb_row = one.tile([1, S], f32, tag="Bb_row", name="Bb_row")
                    nc.vector.tensor_copy(out=Bb_row[:, :], in_=colb[0:1, :])
                    bb_d = dram.tile([1, S], f32, tag="bb_d", name="bb_d")
                    nc.sync.dma_start(out=bb_d[:, :], in_=Bb_row[:, :])
                    Bb_bc = one.tile([CH, S], f32, tag="Bb_bc", name="Bb_bc")
                    nc.gpsimd.dma_start(
                        out=Bb_bc[:, :],
                        in_=bass.AP(tensor=bb_d.tensor, offset=bb_d.offset,
                                    ap=[[0, CH]] + bb_d.ap[1:]))
                    Bblam = one.tile([CH, S], f32, tag="Bblam", name="Bblam")
                    nc.vector.tensor_tensor(
                        out=Bblam[:, :], in0=Bb_bc[:, :], in1=laminv_sb[:, :],
                        op=OP.mult)

                    # sd as per-token columns (PE transpose of bcast rows)
                    sd_col = []
                    for c in range(NCH):
                        trp = ps.tile([128, 128], f32,
                                      tag=("sm" if c % 2 == 0 else "pa"),
                                      name="trpr")
                        nc.tensor.transpose(
                            trp[:, :], sd[:, c * 128:(c + 1) * 128],
                            ident[:, :])
                        col = scr.tile([128, 1], f32, tag="sdc", name="sdc",
                                       bufs=4)
                        nc.vector.reciprocal(out=col[:, :], in_=trp[:, 0:1])
                        sd_col.append(col)

                    # hc, produced chunk-by-chunk so P-matmuls start early
                    hc = [act.tile([128, TSEG], f32r, tag=f"hc{dd}",
                                   name=f"hc{dd}") for dd in range(ND)]
                    for c in range(NCH):
                        for dd in range(ND):
                            eng = nc.vector if dd % 2 == 0 else nc.gpsimd
                            eng.tensor_tensor(
                                out=hc[dd][:, c * 128:(c + 1) * 128],
                                in0=h[dd][:, c * 128:(c + 1) * 128],
                                in1=mu_sb[:, c * 128:(c + 1) * 128],
                                op=OP.subtract)

                    # ---- Bu chunks: v = (hc@BwTw)*laminv*rstd + Bb*laminv ----
                    v_all = act.tile([CH, NCH, S], f32, tag="v_all", name="v_all")
                    for c in range(NCH):
                        bu_ps = ps.tile([128, TSEG], f32,
                                        tag=("pa" if c % 2 == 0 else "pb"),
                                        name="bu_ps")
                        for dd in range(ND):
                            nc.tensor.matmul(
                                bu_ps[:, :S],
                                hc[dd][:, c * 128:(c + 1) * 128],
                                BwTw[dd][:, :],
                                start=(dd == 0), stop=(dd == ND - 1))
                        vt = scr.tile([CH, S], f32, tag="vt", name="vt")
                        nc.vector.tensor_tensor(
                            out=vt[:, :], in0=bu_ps[:, :S], in1=laminv_sb[:, :],
                            op=OP.mult)
                        nc.vector.tensor_scalar_mul(
                            out=vt[:, :], in0=vt[:, :],
                            scalar1=sd_col[c][:, :])
                        nc.vector.tensor_tensor(
                            out=v_all[:, c, :], in0=vt[:, :], in1=Bblam[:, :],
                            op=OP.add)
                        # G partial: column-sum of v chunk
                        gcol = ps.tile([CH, S], f32, tag="gps", name="gcol")
                        nc.tensor.matmul(gcol[0:1, :], U_sb[:, 127:128],
                                         v_all[:, c, :], start=True, stop=True)
                        gt = scr.tile([1, S], f32, tag="gt", name="gt")
                        nc.vector.tensor_tensor(
                            out=gt[:, :], in0=gcol[0:1, :],
                            in1=lamw_sb[0:1, c, :], op=OP.mult)
                        if c == 0:
                            G_sb = one.tile([1, S], f32, tag="G_sb", name="G_sb")
                            nc.vector.tensor_copy(out=G_sb[:, :], in_=gt[:, :])
                        else:
                            nc.vector.tensor_tensor(
                                out=G_sb[:, :], in0=G_sb[:, :], in1=gt[:, :],
                                op=OP.add)

                    # launch AllGather of local-final state ASAP
                    g_in = dram.tile([1, S], f32, tag="g_in", name="g_in")
                    g_out = dram.tile([NCH, S], f32, tag="g_out", name="g_out")
                    nc.sync.dma_start(out=g_in[:, :], in_=G_sb[:, :])
                    nc.gpsimd.collective_compute(
                        "AllGather", mybir.AluOpType.bypass,
                        replica_groups=[[0, 1, 2, 3], [4, 5, 6, 7]],
                        ins=[g_in.opt()], outs=[g_out.opt()],
                    )

                    # ---- intra-chunk cumsums (overlap the collective) ----
                    intra = act.tile([CH, NCH, S], f32, tag="intra", name="intra")
                    for c in range(NCH):
                        cum = ps.tile([CH, S], f32, tag="sm", name="cum")
                        nc.tensor.matmul(cum[:, :], U_sb[:, :], v_all[:, c, :],
                                         start=True, stop=True)
                        nc.vector.tensor_tensor(
                            out=intra[:, c, :], in0=cum[:, :], in1=lamp_sb[:, :],
                            op=OP.mult)

                    # ---- chunk-carry fixup (local) ----
                    S4 = one.tile([NCH, S], f32, tag="S4", name="S4")
                    nc.sync.dma_start(out=S4[:, :], in_=intra[CH - 1:CH, :, :])
                    for c in range(1, NCH):
                        rows = scr.tile([NCH, S], f32, tag="rows", name="rows")
                        nc.vector.tensor_tensor(
                            out=rows[:, :], in0=S4[:, :], in1=lamc_sb[:, c, :],
                            op=OP.mult)
                        pfix = ps.tile([CH, S], f32, tag="sm", name="pfix")
                        nc.tensor.matmul(pfix[:, :], ones4_sb[:, :], rows[:, :],
                                         start=True, stop=True)
                        tmp = scr.tile([CH, S], f32, tag="fixt", name="fixt")
                        nc.vector.tensor_tensor(
                            out=tmp[:, :], in0=pfix[:, :], in1=chc_sb[:, :],
                            op=OP.mult)
                        nc.vector.tensor_tensor(
                            out=intra[:, c, :], in0=intra[:, c, :], in1=tmp[:, :],
                            op=OP.add)

                    if debug and ll == 0:
                        o3 = one.tile([128, NCH * S], f32, tag="dbgcp3",
                                      name="dbgcp3")
                        nc.vector.tensor_copy(
                            out=o3[:, :],
                            in_=intra[:, :, :].rearrange("p a b -> p (a b)"))
                        nc.sync.dma_start(out=dbg["dbg_loc"][:, :], in_=o3[:, :])

                    # ---- transpose hscan -> [s, t] (still during collective) --
                    hsT = [act.tile([128, TSEG], f32r, tag=f"hsT{ss}",
                                    name=f"hsT{ss}") for ss in range(NS)]
                    for c in range(NCH):
                        for ss in range(NS):
                            trp = ps.tile([128, 128], f32,
                                          tag=("sm" if (c * NS + ss) % 2 == 0
                                               else "gps"), name="trp2")
                            nc.tensor.transpose(
                                trp[:, :], intra[:, c, ss * 128:(ss + 1) * 128],
                                ident[:, :])
                            nc.vector.tensor_copy(
                                out=hsT[ss][:, c * 128:(c + 1) * 128],
                                in_=trp[:, :])

                    # ---- cross-core carry: E bcast in [s, t] space ----
                    Gall = one.tile([NCH, S], f32, tag="Gall", name="Gall")
                    nc.sync.dma_start(out=Gall[:, :], in_=g_out[:, :])
                    rowsE = scr.tile([NCH, S], f32, tag="rowsE", name="rowsE")
                    nc.vector.tensor_tensor(
                        out=rowsE[:, :], in0=Gall[:, :], in1=segc_sb[:, :],
                        op=OP.mult)
                    for ss in range(NS):
                        Ebc = ps.tile([128, TSEG], f32,
                                      tag=("pa" if ss == 0 else "pb"), name="Ebc")
                        nc.tensor.matmul(
                            Ebc[:, :], rowsE[:, ss * 128:(ss + 1) * 128],
                            ones4f_sb[:, :], start=True, stop=True)
                        ctmp = scr.tile([128, TSEG], f32, tag="ctmp", name="ctmp")
                        nc.vector.tensor_tensor(
                            out=ctmp[:, :], in0=Ebc[:, :], in1=chc2T_sb[ss][:, :],
                            op=OP.mult)
                        nc.vector.tensor_tensor(
                            out=hsT[ss][:, :], in0=hsT[ss][:, :], in1=ctmp[:, :],
                            op=OP.add)

                    if debug and ll == 0:
                        dump_fm("dbg_hst", hsT)

                    # ---- C projection + residual + Dp*u ----
                    for dd in range(ND):
                        cp_ps = ps4.tile([128, TSEG], f32, tag="acc", name="cp_ps")
                        for ss in range(NS):
                            nc.tensor.matmul(
                                cp_ps[:, :],
                                CwT_sb[ss][:, dd * 128:(dd + 1) * 128],
                                hsT[ss][:, :],
                                start=(ss == 0), stop=(ss == NS - 1))
                        eng = nc.vector if dd < 2 else nc.gpsimd
                        t2 = scr.tile([128, TSEG], f32, tag="t2du", name="t2du")
                        eng.tensor_tensor(
                            out=t2[:, :], in0=hc[dd][:, :], in1=rstd[:, :],
                            op=OP.mult)
                        du = scr.tile([128, TSEG], f32, tag="du", name="du")
                        eng.tensor_scalar(
                            out=du[:, :], in0=t2[:, :],
                            scalar1=lncol("Dpw", ll, dd),
                            scalar2=lncol("Dpb", ll, dd),
                            op0=OP.mult, op1=OP.add)
                        nc.vector.tensor_tensor(
                            out=h[dd][:, :], in0=h[dd][:, :], in1=cp_ps[:, :],
                            op=OP.add)
                        nc.vector.tensor_tensor(
                            out=h[dd][:, :], in0=h[dd][:, :], in1=du[:, :],
                            op=OP.add)

                    if debug and ll == 0:
                        dump_fm("dbg_h1", h)

                    # ---- LN2 ----
                    xn2 = layer_norm("n2w", "n2b", ll, out_tag="xm")

                    # ---- SwiGLU ----
                    sw_ps = [ps4.tile([128, TSEG], f32, tag="acc", name="sw_ps")
                             for _ in range(ND)]
                    f0 = 0
                    for fi, pf in enumerate(FTS):
                        a_ps = ps.tile([128, TSEG], f32, tag="pa", name="a_ps")
                        b_ps = ps.tile([128, TSEG], f32, tag="pb", name="b_ps")
                        for dd in range(ND):
                            nc.tensor.matmul(
                                a_ps[:pf, :], w1sb[dd][:, f0:f0 + pf],
                                xn2[dd][:, :],
                                start=(dd == 0), stop=(dd == ND - 1))
                        for dd in range(ND):
                            nc.tensor.matmul(
                                b_ps[:pf, :], w2sb[dd][:, f0:f0 + pf],
                                xn2[dd][:, :],
                                start=(dd == 0), stop=(dd == ND - 1))
                        sa = scr.tile([128, TSEG], f32r, tag="sa", name="sa")
                        nc.scalar.activation(out=sa[:pf, :], in_=a_ps[:pf, :],
                                             func=AF.Silu)
                        g = scr.tile([128, TSEG], f32r, tag="g", name="g")
                        nc.vector.tensor_tensor(
                            out=g[:pf, :], in0=sa[:pf, :], in1=b_ps[:pf, :],
                            op=OP.mult)
                        w3t = scr.tile([128, D], f32r, tag="w3t", name="w3t")
                        nc.sync.dma_start(
                            out=w3t[:pf, :], in_=d_w3T[ll, f0:f0 + pf, :])
                        for dd in range(ND):
                            nc.tensor.matmul(
                                sw_ps[dd][:, :],
                                w3t[:pf, dd * 128:(dd + 1) * 128],
                                g[:pf, :],
                                start=(fi == 0), stop=(fi == NFT - 1))
                        f0 += pf
                    for dd in range(ND):
                        nc.vector.tensor_tensor(
                            out=h[dd][:, :], in0=h[dd][:, :], in1=sw_ps[dd][:, :],
                            op=OP.add)

            # ---------------- final LN + split AllGather ----------------
            xnf = layer_norm("noww", "nob", None, out_tag="xn")
            if debug:
                dump_fm("dbg_xnf", xnf)

            xf_d = dram.tile([D, TSEG], f32r, tag="xf_d", name="xf_d")
            for dd in range(ND):
                nc.sync.dma_start(
                    out=xf_d[dd * 128:(dd + 1) * 128, :], in_=xnf[dd][:, :])
            xall_d = dram.tile([NCORES * D, TSEG], f32r, tag="xall_d",
                               name="xall_d")
            nc.gpsimd.collective_compute(
                "AllGather", mybir.AluOpType.bypass,
                replica_groups=[list(range(NCORES))],
                ins=[xf_d.opt()], outs=[xall_d.opt()],
            )

            # warmup: own-block head matmuls on local xnf while the AllGather
            # flies (results discarded; keeps the PE HAM-warm)
            scrap_d = dram.tile([128, VC], f32, tag="scrap_d", name="scrap_d")
            with tc.tile_pool(name="whw", bufs=2) as whw:
                wps = ps4.tile([128, TSEG], f32, tag="acc", name="wps")
                nmm = 0
                for vc in range(NVC):
                    hwv = []
                    for dd in range(ND):
                        t = whw.tile([128, VC], f32r, tag=f"whw{dd}",
                                     name=f"whw{dd}")
                        nc.sync.dma_start(
                            out=t[:, :],
                            in_=d_hWT[dd * 128:(dd + 1) * 128,
                                      vc * VC:(vc + 1) * VC])
                        hwv.append(t)
                    for tt in range(NCH):
                        for dd in range(ND):
                            nc.tensor.matmul(
                                wps[:, :VC],
                                xnf[dd][:, tt * 128:(tt + 1) * 128],
                                hwv[dd][:, :],
                                start=(nmm == 0),
                                stop=(nmm == NVC * NCH * ND - 1))
                            nmm += 1
                wsc = scr.tile([128, VC], f32, tag="wsc", name="wsc")
                nc.vector.tensor_copy(out=wsc[:, :], in_=wps[:, :VC])
                nc.sync.dma_start(out=scrap_d[:, :], in_=wsc[:, :])

            act_ctx.__exit__(None, None, None)

            # ---------------- head (vocab-sharded) ----------------
            with tc.tile_pool(name="hd", bufs=1) as hd, \
                 tc.tile_pool(name="hw2", bufs=2) as hw2:
                hbb_sb = hd.tile([128, VSH], f32, tag="hbb", name="hbb")
                nc.sync.dma_start(out=hbb_sb[:, :], in_=d_hbb[:, :])
                xb = []
                for blk in range(NCORES):
                    row = []
                    for dd in range(ND):
                        t = hd.tile([128, TSEG], f32r, tag=f"xb{blk}_{dd}",
                                    name=f"xb{blk}_{dd}")
                        r0 = blk * D + dd * 128
                        nc.sync.dma_start(out=t[:, :], in_=xall_d[r0:r0 + 128, :])
                        row.append(t)
                    xb.append(row)
                for vc in range(NVC):
                    hw = []
                    for dd in range(ND):
                        t = hw2.tile([128, VC], f32r, tag=f"hw{dd}",
                                     name=f"hw{dd}")
                        nc.sync.dma_start(
                            out=t[:, :],
                            in_=d_hWT[dd * 128:(dd + 1) * 128,
                                      vc * VC:(vc + 1) * VC])
                        hw.append(t)
                    for tt in range(NCH):
                        for blk in range(NCORES):
                            hp_ps = ps4.tile([128, TSEG], f32, tag="acc",
                                             name="hp_ps")
                            for dd in range(ND):
                                nc.tensor.matmul(
                                    hp_ps[:, :VC],
                                    xb[blk][dd][:, tt * 128:(tt + 1) * 128],
                                    hw[dd][:, :],
                                    start=(dd == 0), stop=(dd == ND - 1))
                            ot = scr.tile([128, VC], f32, tag="ot", name="ot",
                                          bufs=5)
                            nc.vector.tensor_tensor(
                                out=ot[:, :], in0=hp_ps[:, :VC],
                                in1=hbb_sb[:, vc * VC:(vc + 1) * VC],
                                op=OP.add)
                            t0 = blk * TSEG + tt * 128
                            nc.scalar.dma_start(
                                out=d_out[t0:t0 + 128,
                                          vc * VC:(vc + 1) * VC],
                                in_=ot[:, :])

    nc.compile()
    return nc


def _host_prep(inputs):
    """Build the 8 per-core input maps from full inputs."""
    x = np.asarray(inputs["x"]).astype(np.int32)
    emb = np.asarray(inputs["emb"], np.float32)
    pos = np.asarray(inputs["pos"], np.float32)
    lam = 1.0 / (1.0 + np.exp(-np.asarray(inputs["log_lambda"], np.float64)))
    Bw = np.asarray(inputs["Bw"], np.float32)
    Cw = np.asarray(inputs["Cw"], np.float32)
    w1 = np.asarray(inputs["w1"], np.float32)
    w2 = np.asarray(inputs["w2"], np.float32)
    w3 = np.asarray(inputs["w3"], np.float32)
    headW = np.asarray(inputs["headW"], np.float32)
    headb = np.asarray(inputs["headb"], np.float32)

    BwT = np.ascontiguousarray(Bw.transpose(0, 2, 1))
    CwT = np.ascontiguousarray(Cw.transpose(0, 2, 1))
    w1T = np.ascontiguousarray(w1.transpose(0, 2, 1))
    w2T = np.ascontiguousarray(w2.transpose(0, 2, 1))
    w3T = np.ascontiguousarray(w3.transpose(0, 2, 1))

    # packed LN params:
    # [n1w(L), n1b(L), n2w(L), n2b(L), Dp*n1w(L), now, nob, Dp*n1b(L)]
    Dp = np.asarray(inputs["Dp"], np.float32)
    n1w = np.asarray(inputs["n1w"], np.float32)
    n1b = np.asarray(inputs["n1b"], np.float32)
    lncols = np.zeros((D, NLC), np.float32)
    for i, arr in enumerate((n1w, n1b,
                             np.asarray(inputs["n2w"], np.float32),
                             np.asarray(inputs["n2b"], np.float32),
                             Dp * n1w)):
        lncols[:, i * L:(i + 1) * L] = arr.T
    lncols[:, 5 * L] = np.asarray(inputs["now"], np.float32)
    lncols[:, 5 * L + 1] = np.asarray(inputs["nob"], np.float32)
    lncols[:, 5 * L + 2:6 * L + 2] = (Dp * n1b).T

    i_ar = np.arange(CH, dtype=np.float64)[None, :, None]  # [1, CH, 1]
    lamB = lam[:, None, :]                                 # [L, 1, S]
    laminv = (lamB ** (-i_ar)).astype(np.float32)
    lamp = (lamB ** i_ar).astype(np.float32)
    chc = (lamB ** (i_ar + 1)).astype(np.float32)
    lamw = np.zeros((L, NCH, S), np.float32)
    for c in range(NCH):
        lamw[:, c, :] = (lam ** (TSEG - 1 - CH * c)).astype(np.float32)
    t_ar = np.arange(TSEG, dtype=np.float64)[None, None, :]  # [1, 1, T]
    chc2T = (lam[:, :, None] ** (t_ar + 1)).astype(np.float32)  # [L, S, T]
    lamc = np.zeros((L, NCH, NCH, S), np.float32)
    for c in range(1, NCH):
        for cp in range(c):
            lamc[:, cp, c, :] = (lam ** (CH * (c - 1 - cp))).astype(np.float32)
    U = np.triu(np.ones((CH, CH), np.float32))
    ones4 = np.ones((NCH, 128), np.float32)
    ones4f = np.ones((NCH, TSEG), np.float32)
    ones128r = np.ones((128, 128), np.float32)
    onesD = np.full((128, 128), 1.0 / D, np.float32)

    in_maps = []
    for k in range(NCORES):
        b, r = divmod(k, NSEG)
        t0 = r * TSEG
        segcoef = np.zeros((L, NCH, S), np.float32)
        for sp in range(r):
            segcoef[:, sp, :] = (lam ** (TSEG * (r - 1 - sp))).astype(np.float32)
        v0 = k * VSH
        in_maps.append({
            "x_seg": np.ascontiguousarray(x[b, t0:t0 + TSEG, None]),
            "emb": emb,
            "pos_seg": np.ascontiguousarray(pos[t0:t0 + TSEG]),
            "BwT": BwT, "CwT": CwT, "w1T": w1T, "w2T": w2T, "w3T": w3T,
            "headWT": np.ascontiguousarray(headW[v0:v0 + VSH].T),
            "headb_bc": np.ascontiguousarray(
                np.broadcast_to(headb[v0:v0 + VSH], (128, VSH))),
            "lncols": lncols,
            "laminv": laminv, "lamp": lamp, "chc": chc, "lamw": lamw,
            "chc2T": chc2T, "lamc": lamc, "segcoef": segcoef,
            "Utri": U, "ones4": ones4, "ones4f": ones4f,
            "ones128r": ones128r, "onesD": onesD,
        })
    return in_maps


def kernel(**inputs) -> np.ndarray:
    from concourse.bass_utils import run_bass_kernel_spmd

    if "nc" not in _NC_CACHE:
        _NC_CACHE["nc"] = _build_nc()
    nc = _NC_CACHE["nc"]
    in_maps = _host_prep(inputs)
    res = None
    last_err = None
    for _attempt in range(3):
        try:
            res = run_bass_kernel_spmd(nc, in_maps, core_ids=list(range(NCORES)))
            break
        except Exception as e:  # transient device hiccups: retry
            last_err = e
    if res is None:
        raise last_err
    parts = [res.results[k]["logits"] for k in range(NCORES)]
    full = np.concatenate(parts, axis=1)
    return full.reshape(B, T, V).astype(np.float32)



# revision 19
# speedup vs baseline: 34.6521x; 34.6521x over previous
"""Trainium2 Bass kernel for nn_DiagonalSSM (4-layer diagonal-SSM LM).

Sharding (8 cores):
  - Trunk: token-sharded. Core k handles batch k//4, tokens
    [(k%4)*512, (k%4+1)*512). The SSM scan runs as chunked scaled-cumsums on
    the PE; cross-segment carries use one tiny AllGather (2 groups of 4) per
    layer, launched early (G computed directly from Bu) so it overlaps the
    scan itself.
  - Head: token-sharded. Each core computes logits for its OWN 512 tokens
    over the FULL 32000-vocab (no activation AllGather): lhsT = bf16 head
    weights [128d, 128v], rhs = local xnf [128d, 512t] f32r, PSUM
    [128v, 512t]; bias added on the ACT engine from per-partition columns;
    output written bf16 as [V, 512] (host reassembles with a transposed
    view, which is free).

Layout: activations feature-major ([d, t]); residual stream kept in f32r.
All large matmuls run in float32r (full PE rate, ~13-bit mantissa); the
scan cumsum runs in fp32.
"""

import numpy as np

L, D, S, V = 4, 512, 256, 32000
DFF = 1368
B, T = 2, 2048
NCORES = 8
NSEG = 4
TSEG = 512
NVB = (V + 511) // 512          # 63 head vocab blocks (62 full + 256)
NVT = V // 128                  # 250 head vocab tiles
EPS = 1e-5
CH = 128
NCH = TSEG // CH   # 4
ND = D // 128      # 4
NS = S // 128      # 2
NFT = (DFF + 127) // 128  # 11
FTS = [128] * (DFF // 128) + ([DFF % 128] if DFF % 128 else [])
NLC = 6 * L + 2    # packed LN-param columns

_NC_CACHE = {}


def _build_nc(debug=False):
    import concourse.bass as bass
    import concourse.tile as tile
    from concourse import bacc, mybir
    from concourse.masks import make_identity

    f32 = mybir.dt.float32
    f32r = mybir.dt.float32r
    bf16 = mybir.dt.bfloat16
    i32 = mybir.dt.int32
    AF = mybir.ActivationFunctionType
    OP = mybir.AluOpType

    nc = bacc.Bacc("TRN2", target_bir_lowering=False, debug=False,
                   num_devices=NCORES)

    # ---------------- DRAM I/O ----------------
    d_x = nc.dram_tensor("x_seg", [TSEG, 1], i32, kind="ExternalInput")
    d_emb = nc.dram_tensor("emb", [V, D], f32, kind="ExternalInput")
    d_pos = nc.dram_tensor("pos_seg", [TSEG, D], f32, kind="ExternalInput")
    d_BwT = nc.dram_tensor("BwT", [L, D, S], f32r, kind="ExternalInput")
    d_CwT = nc.dram_tensor("CwT", [L, S, D], f32r, kind="ExternalInput")
    d_w1T = nc.dram_tensor("w1T", [L, D, DFF], f32r, kind="ExternalInput")
    d_w2T = nc.dram_tensor("w2T", [L, D, DFF], f32r, kind="ExternalInput")
    d_w3T = nc.dram_tensor("w3T", [L, DFF, D], f32r, kind="ExternalInput")
    d_hWp = nc.dram_tensor("hWp", [128, 4 * V], bf16, kind="ExternalInput")
    d_hbc = nc.dram_tensor("hbc", [128, NVT], f32, kind="ExternalInput")
    d_lnc = nc.dram_tensor("lncols", [D, NLC], f32, kind="ExternalInput")
    d_laminv = nc.dram_tensor("laminv", [L, CH, S], f32, kind="ExternalInput")
    d_lamp = nc.dram_tensor("lamp", [L, CH, S], f32, kind="ExternalInput")
    d_chc = nc.dram_tensor("chc", [L, CH, S], f32, kind="ExternalInput")
    d_lamw = nc.dram_tensor("lamw", [L, NCH, S], f32, kind="ExternalInput")
    d_chc2T = nc.dram_tensor("chc2T", [L, S, TSEG], f32r, kind="ExternalInput")
    d_lamc = nc.dram_tensor("lamc", [L, NCH, NCH, S], f32, kind="ExternalInput")
    d_segc = nc.dram_tensor("segcoef", [L, NCH, S], f32, kind="ExternalInput")
    d_U = nc.dram_tensor("Utri", [CH, CH], f32, kind="ExternalInput")
    d_ones4 = nc.dram_tensor("ones4", [NCH, 128], f32, kind="ExternalInput")
    d_ones4f = nc.dram_tensor("ones4f", [NCH, TSEG], f32, kind="ExternalInput")
    d_onesD = nc.dram_tensor("onesD", [128, 128], f32r, kind="ExternalInput")

    d_out = nc.dram_tensor("logits", [V, TSEG], bf16, kind="ExternalOutput")

    dbg = {}
    if debug:
        for nm, shp in (
            ("dbg_h0", [D, TSEG]), ("dbg_xn", [D, TSEG]),
            ("dbg_bu", [CH, NCH * S]), ("dbg_loc", [CH, NCH * S]),
            ("dbg_hst", [S, TSEG]), ("dbg_h1", [D, TSEG]),
            ("dbg_xnf", [D, TSEG]),
        ):
            dbg[nm] = nc.dram_tensor(nm, shp, f32, kind="ExternalOutput")

    with tile.TileContext(nc) as tc:
        with (
            tc.tile_pool(name="const", bufs=1) as cpool,
            tc.tile_pool(name="hm", bufs=1) as hm,
            tc.tile_pool(name="scr", bufs=2) as scr,
            tc.tile_pool(name="one", bufs=1) as one,
            tc.tile_pool(name="ps", bufs=1, space="PSUM") as ps,
            tc.tile_pool(name="ps4", bufs=4, space="PSUM") as ps4,
            tc.tile_pool(name="dram", bufs=1, space="DRAM") as dram,
        ):
            # ---------------- constants ----------------
            ident = cpool.tile([128, 128], f32, name="ident")
            make_identity(nc, ident[:, :])
            U_sb = cpool.tile([CH, CH], f32, name="U_sb")
            nc.sync.dma_start(out=U_sb[:, :], in_=d_U[:, :])
            ones4_sb = cpool.tile([NCH, 128], f32, name="ones4_sb")
            nc.sync.dma_start(out=ones4_sb[:, :], in_=d_ones4[:, :])
            ones4f_sb = cpool.tile([NCH, TSEG], f32, name="ones4f_sb")
            nc.sync.dma_start(out=ones4f_sb[:, :], in_=d_ones4f[:, :])
            onesD_sb = cpool.tile([128, 128], f32r, name="onesD_sb")
            nc.sync.dma_start(out=onesD_sb[:, :], in_=d_onesD[:, :])
            eps_sb = cpool.tile([128, 1], f32, name="eps_sb")
            nc.vector.memset(eps_sb[:, :], EPS)
            lnc_sb = []
            for dd in range(ND):
                t = cpool.tile([128, NLC], f32, tag=f"lnc{dd}", name=f"lnc{dd}")
                nc.sync.dma_start(
                    out=t[:, :], in_=d_lnc[dd * 128:(dd + 1) * 128, :])
                lnc_sb.append(t)

            def lncol(key, ll, dd):
                base = {"n1w": 0, "n1b": L, "n2w": 2 * L, "n2b": 3 * L,
                        "Dpw": 4 * L, "Dpb": 5 * L + 2}
                if key == "noww":
                    c = 5 * L
                elif key == "nob":
                    c = 5 * L + 1
                else:
                    c = base[key] + ll
                return lnc_sb[dd][:, c:c + 1]

            # ---------------- h master (feature-major, f32r) --------------
            h = [hm.tile([128, TSEG], f32r, tag=f"h{dd}", name=f"h{dd}")
                 for dd in range(ND)]
            act_ctx = tc.tile_pool(name="act", bufs=1)
            act = act_ctx.__enter__()

            # ---------------- embedding ----------------
            for tt in range(NCH):
                idx_sb = scr.tile([128, 1], i32, tag="idx", name="idx")
                nc.sync.dma_start(
                    out=idx_sb[:, :], in_=d_x[tt * 128:(tt + 1) * 128, :])
                e_t = one.tile([128, D], f32, tag="e_t", name="e_t")
                nc.gpsimd.indirect_dma_start(
                    out=e_t[:, :], out_offset=None, in_=d_emb[:, :],
                    in_offset=bass.IndirectOffsetOnAxis(ap=idx_sb[:, :1], axis=0),
                )
                p_t = one.tile([128, D], f32, tag="p_t", name="p_t")
                nc.sync.dma_start(
                    out=p_t[:, :], in_=d_pos[tt * 128:(tt + 1) * 128, :])
                htm = one.tile([128, D], f32, tag="htm", name="htm")
                nc.vector.tensor_tensor(
                    out=htm[:, :], in0=e_t[:, :], in1=p_t[:, :], op=OP.add)
                for dd in range(ND):
                    trp = ps.tile([128, 128], f32,
                                  tag=("sm" if dd % 2 == 0 else "gps"), name="trp")
                    nc.tensor.transpose(
                        trp[:, :], htm[:, dd * 128:(dd + 1) * 128], ident[:, :])
                    nc.vector.tensor_copy(
                        out=h[dd][:, tt * 128:(tt + 1) * 128], in_=trp[:, :])

            def dump_fm(key, tiles):
                for dd in range(len(tiles)):
                    o = one.tile([128, TSEG], f32, tag="dbgcp", name="dbgcp")
                    nc.vector.tensor_copy(out=o[:, :], in_=tiles[dd][:, :])
                    nc.sync.dma_start(
                        out=dbg[key][dd * 128:(dd + 1) * 128, :], in_=o[:, :])

            if debug:
                dump_fm("dbg_h0", h)

            # ---------------- LN helper ----------------
            def ln_stats():
                # mean and E[x^2] accumulate in parallel (separate banks)
                mu = ps.tile([128, TSEG], f32, tag="sm", name="mu")
                ex2 = ps.tile([128, TSEG], f32, tag="gps", name="ex2")
                for dd in range(ND):
                    nc.tensor.matmul(mu[:, :], onesD_sb[:, :], h[dd][:, :],
                                     start=(dd == 0), stop=(dd == ND - 1))
                for dd in range(ND):
                    h2 = scr.tile([128, TSEG], f32r, tag="h2", name="h2")
                    nc.vector.tensor_tensor(
                        out=h2[:, :], in0=h[dd][:, :], in1=h[dd][:, :], op=OP.mult)
                    nc.tensor.matmul(ex2[:, :], onesD_sb[:, :], h2[:, :],
                                     start=(dd == 0), stop=(dd == ND - 1))
                mu_sb = one.tile([128, TSEG], f32, tag="mu_sb", name="mu_sb")
                nc.vector.tensor_copy(out=mu_sb[:, :], in_=mu[:, :])
                var = one.tile([128, TSEG], f32, tag="var_sb", name="var_sb")
                nc.vector.tensor_tensor(
                    out=var[:, :], in0=mu_sb[:, :], in1=mu[:, :], op=OP.mult)
                nc.vector.tensor_tensor(
                    out=var[:, :], in0=ex2[:, :], in1=var[:, :], op=OP.subtract)
                sd = one.tile([128, TSEG], f32, tag="sd", name="sd")
                nc.scalar.activation(out=sd[:, :], in_=var[:, :], func=AF.Sqrt,
                                     bias=eps_sb[:, :], scale=1.0)
                rstd = one.tile([128, TSEG], f32, tag="rstd", name="rstd")
                nc.vector.reciprocal(out=rstd[:, :], in_=sd[:, :])
                return mu_sb, sd, rstd

            def layer_norm(w_key, b_key, ll=None, out_tag="xn", out_dtype=None):
                mu_sb, sd, rstd = ln_stats()
                hc = []
                for dd in range(ND):
                    c = act.tile([128, TSEG], f32r, tag=f"hc{dd}", name=f"hc{dd}")
                    eng = nc.vector if dd < 2 else nc.gpsimd
                    eng.tensor_tensor(
                        out=c[:, :], in0=h[dd][:, :], in1=mu_sb[:, :],
                        op=OP.subtract)
                    hc.append(c)
                xn = []
                for dd in range(ND):
                    eng = nc.vector if dd < 2 else nc.gpsimd
                    t1 = scr.tile([128, TSEG], f32, tag="lnt1", name="lnt1")
                    eng.tensor_tensor(
                        out=t1[:, :], in0=hc[dd][:, :], in1=rstd[:, :], op=OP.mult)
                    xo = act.tile([128, TSEG], out_dtype or f32r,
                                  tag=f"{out_tag}{dd}", name=f"{out_tag}{dd}")
                    eng.tensor_scalar(
                        out=xo[:, :], in0=t1[:, :],
                        scalar1=lncol(w_key, ll, dd), scalar2=lncol(b_key, ll, dd),
                        op0=OP.mult, op1=OP.add)
                    xn.append(xo)
                return xn

            # ---------------- layers ----------------
            with (
                tc.tile_pool(name="tabs", bufs=1) as tabs,
                tc.tile_pool(name="wbig", bufs=1) as wbig,
            ):
                for ll in range(L):
                    laminv_sb = tabs.tile([CH, S], f32, tag="laminv", name="laminv")
                    lamp_sb = tabs.tile([CH, S], f32, tag="lamp", name="lamp")
                    chc_sb = tabs.tile([CH, S], f32, tag="chc", name="chc")
                    lamw_sb = tabs.tile([1, NCH, S], f32, tag="lamw", name="lamw")
                    chc2T_sb = [tabs.tile([128, TSEG], f32, tag=f"chc2T{ss}",
                                          name=f"chc2T{ss}") for ss in range(NS)]
                    lamc_sb = tabs.tile([NCH, NCH, S], f32, tag="lamc", name="lamc")
                    segc_sb = tabs.tile([NCH, S], f32, tag="segc", name="segc")
                    nc.sync.dma_start(out=laminv_sb[:, :], in_=d_laminv[ll])
                    nc.sync.dma_start(out=lamp_sb[:, :], in_=d_lamp[ll])
                    nc.sync.dma_start(out=chc_sb[:, :], in_=d_chc[ll])
                    nc.sync.dma_start(out=lamw_sb[0:1, :, :], in_=d_lamw[ll])
                    for ss in range(NS):
                        nc.sync.dma_start(
                            out=chc2T_sb[ss][:, :],
                            in_=d_chc2T[ll, ss * 128:(ss + 1) * 128, :])
                    nc.sync.dma_start(out=lamc_sb[:, :, :], in_=d_lamc[ll])
                    nc.sync.dma_start(out=segc_sb[:, :], in_=d_segc[ll])
                    BwT_sb = []
                    for dd in range(ND):
                        t = tabs.tile([128, S], f32r, tag=f"BwT{dd}",
                                      name=f"BwT{dd}")
                        nc.sync.dma_start(
                            out=t[:, :], in_=d_BwT[ll, dd * 128:(dd + 1) * 128, :])
                        BwT_sb.append(t)
                    CwT_sb = []
                    for ss in range(NS):
                        t = tabs.tile([128, D], f32r, tag=f"CwT{ss}",
                                      name=f"CwT{ss}")
                        nc.sync.dma_start(
                            out=t[:, :], in_=d_CwT[ll, ss * 128:(ss + 1) * 128, :])
                        CwT_sb.append(t)
                    w1sb, w2sb = [], []
                    for dd in range(ND):
                        t = wbig.tile([128, DFF], f32r, tag=f"w1sb{dd}",
                                      name=f"w1sb{dd}")
                        nc.sync.dma_start(
                            out=t[:, :], in_=d_w1T[ll, dd * 128:(dd + 1) * 128, :])
                        w1sb.append(t)
                        t = wbig.tile([128, DFF], f32r, tag=f"w2sb{dd}",
                                      name=f"w2sb{dd}")
                        nc.sync.dma_start(
                            out=t[:, :], in_=d_w2T[ll, dd * 128:(dd + 1) * 128, :])
                        w2sb.append(t)

                    # per-layer folded Bw weights (off critical path)
                    BwTw = []
                    for dd in range(ND):
                        t = tabs.tile([128, S], f32r, tag=f"BwTw{dd}",
                                      name=f"BwTw{dd}")
                        nc.gpsimd.tensor_scalar_mul(
                            out=t[:, :], in0=BwT_sb[dd][:, :],
                            scalar1=lncol("n1w", ll, dd))
                        BwTw.append(t)

                    # ---- LN1 stats; P-matmuls start per-chunk on hc ----
                    mu_sb, sd, rstd = ln_stats()
                    # Bb[s] = sum_d n1b[d] BwT[d,s]; broadcast + pre-scale
                    n1br = []
                    for dd in range(ND):
                        t = scr.tile([128, 1], f32r, tag="n1br", name="n1br")
                        nc.vector.tensor_copy(out=t[:, :],
                                              in_=lncol("n1b", ll, dd))
                        n1br.append(t)
                    colb = ps.tile([CH, S], f32, tag="gps", name="colb")
                    for dd in range(ND):
                        nc.tensor.matmul(colb[0:1, :], n1br[dd][:, :],
                                         BwT_sb[dd][:, :],
                                         start=(dd == 0), stop=(dd == ND - 1))
                    Bb_row = one.tile([1, S], f32, tag="Bb_row", name="Bb_row")
                    nc.vector.tensor_copy(out=Bb_row[:, :], in_=colb[0:1, :])
                    bb_d = dram.tile([1, S], f32, tag="bb_d", name="bb_d")
                    nc.sync.dma_start(out=bb_d[:, :], in_=Bb_row[:, :])
                    Bb_bc = one.tile([CH, S], f32, tag="Bb_bc", name="Bb_bc")
                    nc.gpsimd.dma_start(
                        out=Bb_bc[:, :],
                        in_=bass.AP(tensor=bb_d.tensor, offset=bb_d.offset,
                                    ap=[[0, CH]] + bb_d.ap[1:]))
                    Bblam = one.tile([CH, S], f32, tag="Bblam", name="Bblam")
                    nc.vector.tensor_tensor(
                        out=Bblam[:, :], in0=Bb_bc[:, :], in1=laminv_sb[:, :],
                        op=OP.mult)

                    # sd as per-token columns (PE transpose of bcast rows)
                    sd_col = []
                    for c in range(NCH):
                        trp = ps.tile([128, 128], f32,
                                      tag=("sm" if c % 2 == 0 else "pa"),
                                      name="trpr")
                        nc.tensor.transpose(
                            trp[:, :], sd[:, c * 128:(c + 1) * 128],
                            ident[:, :])
                        col = scr.tile([128, 1], f32, tag="sdc", name="sdc",
                                       bufs=4)
                        nc.vector.reciprocal(out=col[:, :], in_=trp[:, 0:1])
                        sd_col.append(col)

                    # hc, produced chunk-by-chunk so P-matmuls start early
                    hc = [act.tile([128, TSEG], f32r, tag=f"hc{dd}",
                                   name=f"hc{dd}") for dd in range(ND)]
                    for c in range(NCH):
                        for dd in range(ND):
                            eng = nc.vector if dd % 2 == 0 else nc.gpsimd
                            eng.tensor_tensor(
                                out=hc[dd][:, c * 128:(c + 1) * 128],
                                in0=h[dd][:, c * 128:(c + 1) * 128],
                                in1=mu_sb[:, c * 128:(c + 1) * 128],
                                op=OP.subtract)

                    # ---- Bu chunks: v = (hc@BwTw)*laminv*rstd + Bb*laminv ----
                    v_all = act.tile([CH, NCH, S], f32, tag="v_all", name="v_all")
                    for c in range(NCH):
                        bu_ps = ps.tile([128, TSEG], f32,
                                        tag=("pa" if c % 2 == 0 else "pb"),
                                        name="bu_ps")
                        for dd in range(ND):
                            nc.tensor.matmul(
                                bu_ps[:, :S],
                                hc[dd][:, c * 128:(c + 1) * 128],
                                BwTw[dd][:, :],
                                start=(dd == 0), stop=(dd == ND - 1))
                        vt = scr.tile([CH, S], f32, tag="vt", name="vt")
                        nc.vector.tensor_tensor(
                            out=vt[:, :], in0=bu_ps[:, :S], in1=laminv_sb[:, :],
                            op=OP.mult)
                        nc.vector.tensor_scalar_mul(
                            out=vt[:, :], in0=vt[:, :],
                            scalar1=sd_col[c][:, :])
                        nc.vector.tensor_tensor(
                            out=v_all[:, c, :], in0=vt[:, :], in1=Bblam[:, :],
                            op=OP.add)
                        # G partial: column-sum of v chunk
                        gcol = ps.tile([CH, S], f32, tag="gps", name="gcol")
                        nc.tensor.matmul(gcol[0:1, :], U_sb[:, 127:128],
                                         v_all[:, c, :], start=True, stop=True)
                        gt = scr.tile([1, S], f32, tag="gt", name="gt")
                        nc.vector.tensor_tensor(
                            out=gt[:, :], in0=gcol[0:1, :],
                            in1=lamw_sb[0:1, c, :], op=OP.mult)
                        if c == 0:
                            G_sb = one.tile([1, S], f32, tag="G_sb", name="G_sb")
                            nc.vector.tensor_copy(out=G_sb[:, :], in_=gt[:, :])
                        else:
                            nc.vector.tensor_tensor(
                                out=G_sb[:, :], in0=G_sb[:, :], in1=gt[:, :],
                                op=OP.add)

                    # launch AllGather of local-final state ASAP
                    g_in = dram.tile([1, S], f32, tag="g_in", name="g_in")
                    g_out = dram.tile([NCH, S], f32, tag="g_out", name="g_out")
                    nc.sync.dma_start(out=g_in[:, :], in_=G_sb[:, :])
                    nc.gpsimd.collective_compute(
                        "AllGather", mybir.AluOpType.bypass,
                        replica_groups=[[0, 1, 2, 3], [4, 5, 6, 7]],
                        ins=[g_in.opt()], outs=[g_out.opt()],
                    )

                    # ---- intra-chunk cumsums (overlap the collective) ----
                    intra = act.tile([CH, NCH, S], f32, tag="intra", name="intra")
                    for c in range(NCH):
                        cum = ps.tile([CH, S], f32, tag="sm", name="cum")
                        nc.tensor.matmul(cum[:, :], U_sb[:, :], v_all[:, c, :],
                                         start=True, stop=True)
                        nc.vector.tensor_tensor(
                            out=intra[:, c, :], in0=cum[:, :], in1=lamp_sb[:, :],
                            op=OP.mult)

                    # ---- chunk-carry fixup (local) ----
                    S4 = one.tile([NCH, S], f32, tag="S4", name="S4")
                    nc.sync.dma_start(out=S4[:, :], in_=intra[CH - 1:CH, :, :])
                    for c in range(1, NCH):
                        rows = scr.tile([NCH, S], f32, tag="rows", name="rows")
                        nc.vector.tensor_tensor(
                            out=rows[:, :], in0=S4[:, :], in1=lamc_sb[:, c, :],
                            op=OP.mult)
                        pfix = ps.tile([CH, S], f32, tag="sm", name="pfix")
                        nc.tensor.matmul(pfix[:, :], ones4_sb[:, :], rows[:, :],
                                         start=True, stop=True)
                        tmp = scr.tile([CH, S], f32, tag="fixt", name="fixt")
                        nc.vector.tensor_tensor(
                            out=tmp[:, :], in0=pfix[:, :], in1=chc_sb[:, :],
                            op=OP.mult)
                        nc.vector.tensor_tensor(
                            out=intra[:, c, :], in0=intra[:, c, :], in1=tmp[:, :],
                            op=OP.add)

                    if debug and ll == 0:
                        o3 = one.tile([128, NCH * S], f32, tag="dbgcp3",
                                      name="dbgcp3")
                        nc.vector.tensor_copy(
                            out=o3[:, :],
                            in_=intra[:, :, :].rearrange("p a b -> p (a b)"))
                        nc.sync.dma_start(out=dbg["dbg_loc"][:, :], in_=o3[:, :])

                    # ---- transpose hscan -> [s, t] (still during collective) --
                    hsT = [act.tile([128, TSEG], f32r, tag=f"hsT{ss}",
                                    name=f"hsT{ss}") for ss in range(NS)]
                    for c in range(NCH):
                        for ss in range(NS):
                            trp = ps.tile([128, 128], f32,
                                          tag=("sm" if (c * NS + ss) % 2 == 0
                                               else "gps"), name="trp2")
                            nc.tensor.transpose(
                                trp[:, :], intra[:, c, ss * 128:(ss + 1) * 128],
                                ident[:, :])
                            nc.vector.tensor_copy(
                                out=hsT[ss][:, c * 128:(c + 1) * 128],
                                in_=trp[:, :])

                    # ---- cross-core carry: E bcast in [s, t] space ----
                    Gall = one.tile([NCH, S], f32, tag="Gall", name="Gall")
                    nc.sync.dma_start(out=Gall[:, :], in_=g_out[:, :])
                    rowsE = scr.tile([NCH, S], f32, tag="rowsE", name="rowsE")
                    nc.vector.tensor_tensor(
                        out=rowsE[:, :], in0=Gall[:, :], in1=segc_sb[:, :],
                        op=OP.mult)
                    for ss in range(NS):
                        Ebc = ps.tile([128, TSEG], f32,
                                      tag=("pa" if ss == 0 else "pb"), name="Ebc")
                        nc.tensor.matmul(
                            Ebc[:, :], rowsE[:, ss * 128:(ss + 1) * 128],
                            ones4f_sb[:, :], start=True, stop=True)
                        ctmp = scr.tile([128, TSEG], f32, tag="ctmp", name="ctmp")
                        nc.vector.tensor_tensor(
                            out=ctmp[:, :], in0=Ebc[:, :], in1=chc2T_sb[ss][:, :],
                            op=OP.mult)
                        nc.vector.tensor_tensor(
                            out=hsT[ss][:, :], in0=hsT[ss][:, :], in1=ctmp[:, :],
                            op=OP.add)

                    if debug and ll == 0:
                        dump_fm("dbg_hst", hsT)

                    # ---- C projection + residual + Dp*u ----
                    for dd in range(ND):
                        cp_ps = ps4.tile([128, TSEG], f32, tag="acc", name="cp_ps")
                        for ss in range(NS):
                            nc.tensor.matmul(
                                cp_ps[:, :],
                                CwT_sb[ss][:, dd * 128:(dd + 1) * 128],
                                hsT[ss][:, :],
                                start=(ss == 0), stop=(ss == NS - 1))
                        eng = nc.vector if dd < 2 else nc.gpsimd
                        t2 = scr.tile([128, TSEG], f32, tag="t2du", name="t2du")
                        eng.tensor_tensor(
                            out=t2[:, :], in0=hc[dd][:, :], in1=rstd[:, :],
                            op=OP.mult)
                        du = scr.tile([128, TSEG], f32, tag="du", name="du")
                        eng.tensor_scalar(
                            out=du[:, :], in0=t2[:, :],
                            scalar1=lncol("Dpw", ll, dd),
                            scalar2=lncol("Dpb", ll, dd),
                            op0=OP.mult, op1=OP.add)
                        nc.vector.tensor_tensor(
                            out=h[dd][:, :], in0=h[dd][:, :], in1=cp_ps[:, :],
                            op=OP.add)
                        nc.vector.tensor_tensor(
                            out=h[dd][:, :], in0=h[dd][:, :], in1=du[:, :],
                            op=OP.add)

                    if debug and ll == 0:
                        dump_fm("dbg_h1", h)

                    # ---- LN2 ----
                    xn2 = layer_norm("n2w", "n2b", ll, out_tag="xm")

                    # ---- SwiGLU ----
                    sw_ps = [ps4.tile([128, TSEG], f32, tag="acc", name="sw_ps")
                             for _ in range(ND)]
                    f0 = 0
                    for fi, pf in enumerate(FTS):
                        a_ps = ps.tile([128, TSEG], f32, tag="pa", name="a_ps")
                        b_ps = ps.tile([128, TSEG], f32, tag="pb", name="b_ps")
                        for dd in range(ND):
                            nc.tensor.matmul(
                                a_ps[:pf, :], w1sb[dd][:, f0:f0 + pf],
                                xn2[dd][:, :],
                                start=(dd == 0), stop=(dd == ND - 1))
                        for dd in range(ND):
                            nc.tensor.matmul(
                                b_ps[:pf, :], w2sb[dd][:, f0:f0 + pf],
                                xn2[dd][:, :],
                                start=(dd == 0), stop=(dd == ND - 1))
                        sa = scr.tile([128, TSEG], f32r, tag="sa", name="sa")
                        nc.scalar.activation(out=sa[:pf, :], in_=a_ps[:pf, :],
                                             func=AF.Silu)
                        g = scr.tile([128, TSEG], f32r, tag="g", name="g")
                        nc.vector.tensor_tensor(
                            out=g[:pf, :], in0=sa[:pf, :], in1=b_ps[:pf, :],
                            op=OP.mult)
                        w3t = scr.tile([128, D], f32r, tag="w3t", name="w3t")
                        nc.sync.dma_start(
                            out=w3t[:pf, :], in_=d_w3T[ll, f0:f0 + pf, :])
                        for dd in range(ND):
                            nc.tensor.matmul(
                                sw_ps[dd][:, :],
                                w3t[:pf, dd * 128:(dd + 1) * 128],
                                g[:pf, :],
                                start=(fi == 0), stop=(fi == NFT - 1))
                        f0 += pf
                    for dd in range(ND):
                        nc.vector.tensor_tensor(
                            out=h[dd][:, :], in0=h[dd][:, :], in1=sw_ps[dd][:, :],
                            op=OP.add)

            # ---------------- final LN (bf16 out for the bf16 head) ------
            xnf = layer_norm("noww", "nob", None, out_tag="xn",
                             out_dtype=bf16)
            if debug:
                dump_fm("dbg_xnf", xnf)

            # ---------------- head (token-sharded, full vocab) -----------
            # Per vocab block vb: one batched DMA loads [128d, 4dd x nv] of
            # bf16 weights; nv/128 PSUM groups of [128v, 512t]; bias+copy on
            # the ACT engine; one batched DMA writes [nv, 512] bf16 logits.
            with tc.tile_pool(name="hd", bufs=1) as hd, \
                 tc.tile_pool(name="hw2", bufs=3) as hw2, \
                 tc.tile_pool(name="ho", bufs=3) as ho:
                hbc_sb = hd.tile([128, NVT], f32, tag="hbc", name="hbc")
                nc.sync.dma_start(out=hbc_sb[:, :], in_=d_hbc[:, :])
                for vb in range(NVB):
                    v0 = vb * 512
                    nv = min(512, V - v0)
                    nsub = nv // 128
                    wt = hw2.tile([128, 4 * 512], bf16, tag="wt", name="wt")
                    hw_ap = d_hWp[:, :]
                    nc.sync.dma_start(
                        out=wt[:, :4 * nv],
                        in_=bass.AP(tensor=hw_ap.tensor,
                                    offset=hw_ap.offset + v0,
                                    ap=[[4 * V, 128], [V, 4], [1, nv]]))
                    ot = ho.tile([128, 4 * TSEG], bf16, tag="ot", name="ot")
                    for sub in range(nsub):
                        hp_ps = ps4.tile([128, TSEG], f32, tag="acc",
                                         name="hp_ps")
                        for dd in range(ND):
                            nc.tensor.matmul(
                                hp_ps[:, :],
                                wt[:, dd * nv + sub * 128:
                                   dd * nv + sub * 128 + 128],
                                xnf[dd][:, :],
                                start=(dd == 0), stop=(dd == ND - 1))
                        nc.scalar.activation(
                            out=ot[:, sub * TSEG:(sub + 1) * TSEG],
                            in_=hp_ps[:, :], func=AF.Identity,
                            bias=hbc_sb[:, vb * 4 + sub:vb * 4 + sub + 1],
                            scale=1.0)
                    out_ap = d_out[:, :]
                    nc.sync.dma_start(
                        out=bass.AP(tensor=out_ap.tensor,
                                    offset=out_ap.offset + v0 * TSEG,
                                    ap=[[TSEG, 128], [128 * TSEG, nsub],
                                        [1, TSEG]]),
                        in_=ot[:, :nsub * TSEG])

            act_ctx.__exit__(None, None, None)

    nc.compile()
    return nc


def _host_prep(inputs):
    """Build the 8 per-core input maps from full inputs."""
    x = np.asarray(inputs["x"]).astype(np.int32)
    emb = np.asarray(inputs["emb"], np.float32)
    pos = np.asarray(inputs["pos"], np.float32)
    lam = 1.0 / (1.0 + np.exp(-np.asarray(inputs["log_lambda"], np.float64)))
    Bw = np.asarray(inputs["Bw"], np.float32)
    Cw = np.asarray(inputs["Cw"], np.float32)
    w1 = np.asarray(inputs["w1"], np.float32)
    w2 = np.asarray(inputs["w2"], np.float32)
    w3 = np.asarray(inputs["w3"], np.float32)
    headW = np.asarray(inputs["headW"], np.float32)
    headb = np.asarray(inputs["headb"], np.float32)

    BwT = np.ascontiguousarray(Bw.transpose(0, 2, 1))
    CwT = np.ascontiguousarray(Cw.transpose(0, 2, 1))
    w1T = np.ascontiguousarray(w1.transpose(0, 2, 1))
    w2T = np.ascontiguousarray(w2.transpose(0, 2, 1))
    w3T = np.ascontiguousarray(w3.transpose(0, 2, 1))

    import ml_dtypes
    # hWp[p, dd*V + v] = headW[v, dd*128 + p]  (bf16)
    hWp = np.ascontiguousarray(
        headW.T.reshape(ND, 128, V).transpose(1, 0, 2).reshape(128, 4 * V)
    ).astype(ml_dtypes.bfloat16)
    # hbc[p, t] = headb[t*128 + p]
    hbc = np.ascontiguousarray(headb.reshape(NVT, 128).T)

    # packed LN params:
    # [n1w(L), n1b(L), n2w(L), n2b(L), Dp*n1w(L), now, nob, Dp*n1b(L)]
    Dp = np.asarray(inputs["Dp"], np.float32)
    n1w = np.asarray(inputs["n1w"], np.float32)
    n1b = np.asarray(inputs["n1b"], np.float32)
    lncols = np.zeros((D, NLC), np.float32)
    for i, arr in enumerate((n1w, n1b,
                             np.asarray(inputs["n2w"], np.float32),
                             np.asarray(inputs["n2b"], np.float32),
                             Dp * n1w)):
        lncols[:, i * L:(i + 1) * L] = arr.T
    lncols[:, 5 * L] = np.asarray(inputs["now"], np.float32)
    lncols[:, 5 * L + 1] = np.asarray(inputs["nob"], np.float32)
    lncols[:, 5 * L + 2:6 * L + 2] = (Dp * n1b).T

    i_ar = np.arange(CH, dtype=np.float64)[None, :, None]  # [1, CH, 1]
    lamB = lam[:, None, :]                                 # [L, 1, S]
    laminv = (lamB ** (-i_ar)).astype(np.float32)
    lamp = (lamB ** i_ar).astype(np.float32)
    chc = (lamB ** (i_ar + 1)).astype(np.float32)
    lamw = np.zeros((L, NCH, S), np.float32)
    for c in range(NCH):
        lamw[:, c, :] = (lam ** (TSEG - 1 - CH * c)).astype(np.float32)
    t_ar = np.arange(TSEG, dtype=np.float64)[None, None, :]  # [1, 1, T]
    chc2T = (lam[:, :, None] ** (t_ar + 1)).astype(np.float32)  # [L, S, T]
    lamc = np.zeros((L, NCH, NCH, S), np.float32)
    for c in range(1, NCH):
        for cp in range(c):
            lamc[:, cp, c, :] = (lam ** (CH * (c - 1 - cp))).astype(np.float32)
    U = np.triu(np.ones((CH, CH), np.float32))
    ones4 = np.ones((NCH, 128), np.float32)
    ones4f = np.ones((NCH, TSEG), np.float32)
    onesD = np.full((128, 128), 1.0 / D, np.float32)

    in_maps = []
    for k in range(NCORES):
        b, r = divmod(k, NSEG)
        t0 = r * TSEG
        segcoef = np.zeros((L, NCH, S), np.float32)
        for sp in range(r):
            segcoef[:, sp, :] = (lam ** (TSEG * (r - 1 - sp))).astype(np.float32)
        in_maps.append({
            "x_seg": np.ascontiguousarray(x[b, t0:t0 + TSEG, None]),
            "emb": emb,
            "pos_seg": np.ascontiguousarray(pos[t0:t0 + TSEG]),
            "BwT": BwT, "CwT": CwT, "w1T": w1T, "w2T": w2T, "w3T": w3T,
            "hWp": hWp, "hbc": hbc,
            "lncols": lncols,
            "laminv": laminv, "lamp": lamp, "chc": chc, "lamw": lamw,
            "chc2T": chc2T, "lamc": lamc, "segcoef": segcoef,
            "Utri": U, "ones4": ones4, "ones4f": ones4f,
            "onesD": onesD,
        })
    return in_maps


def kernel(**inputs) -> np.ndarray:
    from concourse.bass_utils import run_bass_kernel_spmd

    if "nc" not in _NC_CACHE:
        _NC_CACHE["nc"] = _build_nc()
    nc = _NC_CACHE["nc"]
    in_maps = _host_prep(inputs)
    res = None
    last_err = None
    for _attempt in range(3):
        try:
            res = run_bass_kernel_spmd(nc, in_maps, core_ids=list(range(NCORES)))
            break
        except Exception as e:  # transient device hiccups: retry
            last_err = e
    if res is None:
        raise last_err
    return assemble_output([res.results[k]["logits"] for k in range(NCORES)])


def assemble_output(parts):
    """parts[k]: [V, TSEG] bf16 logits for core k's tokens -> [B, T, V] f32."""
    out = np.empty((B, T, V), np.float32)
    for k in range(NCORES):
        b, r = divmod(k, NSEG)
        out[b, r * TSEG:(r + 1) * TSEG] = parts[k].astype(np.float32).T
    return out



# revision 37
# speedup vs baseline: 43.0925x; 1.2436x over previous
"""Trainium2 Bass kernel for nn_DiagonalSSM (4-layer diagonal-SSM LM).

Sharding (8 cores):
  - Trunk: token-sharded. Core k handles batch k//4, tokens
    [(k%4)*512, (k%4+1)*512). The SSM scan runs as chunked scaled-cumsums on
    the PE; cross-segment carries use one tiny AllGather (2 groups of 4) per
    layer, launched early (G computed directly from Bu) so it overlaps the
    scan itself.
  - Head: token-sharded. Each core computes logits for its OWN 512 tokens
    over the FULL 32000-vocab (no activation AllGather): lhsT = bf16 head
    weights [128d, 128v], rhs = local xnf [128d, 512t] f32r, PSUM
    [128v, 512t]; bias added on the ACT engine from per-partition columns;
    output written bf16 as [V, 512] (host reassembles with a transposed
    view, which is free).

Layout: activations feature-major ([d, t]); residual stream kept in f32r.
All large matmuls run in float32r (full PE rate, ~13-bit mantissa); the
scan cumsum runs in fp32.
"""

import numpy as np

L, D, S, V = 4, 512, 256, 32000
DFF = 1368
B, T = 2, 2048
NCORES = 8
NSEG = 4
TSEG = 512
NVB = (V + 511) // 512          # 63 head vocab blocks (62 full + 256)
NVT = V // 128                  # 250 head vocab tiles
EPS = 1e-5
CH = 128
NCH = TSEG // CH   # 4
ND = D // 128      # 4
NS = S // 128      # 2
NFT = (DFF + 127) // 128  # 11
FTS = [128] * (DFF // 128) + ([DFF % 128] if DFF % 128 else [])
NLC = 6 * L + 2    # packed LN-param columns

_NC_CACHE = {}


def _build_nc(debug=False):
    import concourse.bass as bass
    import concourse.tile as tile
    from concourse import bacc, mybir
    from concourse.masks import make_identity

    f32 = mybir.dt.float32
    f32r = mybir.dt.float32r
    bf16 = mybir.dt.bfloat16
    i32 = mybir.dt.int32
    AF = mybir.ActivationFunctionType
    OP = mybir.AluOpType

    nc = bacc.Bacc("TRN2", target_bir_lowering=False, debug=False,
                   num_devices=NCORES)

    # ---------------- DRAM I/O ----------------
    d_x = nc.dram_tensor("x_seg", [TSEG, 1], i32, kind="ExternalInput")
    d_emb = nc.dram_tensor("emb", [V, D], f32, kind="ExternalInput")
    d_pos = nc.dram_tensor("pos_seg", [TSEG, D], f32, kind="ExternalInput")
    d_BwT = nc.dram_tensor("BwT", [L, D, S], f32r, kind="ExternalInput")
    d_CwT = nc.dram_tensor("CwT", [L, S, D], f32r, kind="ExternalInput")
    d_w1T = nc.dram_tensor("w1T", [L, D, DFF], f32r, kind="ExternalInput")
    d_w2T = nc.dram_tensor("w2T", [L, D, DFF], f32r, kind="ExternalInput")
    d_w3T = nc.dram_tensor("w3T", [L, DFF, D], f32r, kind="ExternalInput")
    d_hWp = nc.dram_tensor("hWp", [128, 4 * V], bf16, kind="ExternalInput")
    d_hbc = nc.dram_tensor("hbc", [128, NVT], f32, kind="ExternalInput")
    d_lnc = nc.dram_tensor("lncols", [D, NLC], f32, kind="ExternalInput")
    d_laminv = nc.dram_tensor("laminv", [L, CH, S], f32, kind="ExternalInput")
    d_lamp = nc.dram_tensor("lamp", [L, CH, S], f32, kind="ExternalInput")
    d_chc = nc.dram_tensor("chc", [L, CH, S], f32, kind="ExternalInput")
    d_lamw = nc.dram_tensor("lamw", [L, NCH, S], f32, kind="ExternalInput")
    d_chc2T = nc.dram_tensor("chc2T", [L, S, TSEG], f32, kind="ExternalInput")
    d_lamc = nc.dram_tensor("lamc", [L, NCH, NCH, S], f32, kind="ExternalInput")
    d_segc = nc.dram_tensor("segcoef", [L, NCH, S], f32, kind="ExternalInput")
    d_U = nc.dram_tensor("Utri", [CH, CH], f32, kind="ExternalInput")
    d_ones4 = nc.dram_tensor("ones4", [NCH, 128], f32, kind="ExternalInput")
    d_ones4f = nc.dram_tensor("ones4f", [NCH, TSEG], f32, kind="ExternalInput")
    d_onesD = nc.dram_tensor("onesD", [128, 128], f32r, kind="ExternalInput")

    d_out = nc.dram_tensor("logits", [V, TSEG], bf16, kind="ExternalOutput")

    dbg = {}
    if debug:
        for nm, shp in (
            ("dbg_h0", [D, TSEG]), ("dbg_xn", [D, TSEG]),
            ("dbg_bu", [CH, NCH * S]), ("dbg_loc", [CH, NCH * S]),
            ("dbg_hst", [S, TSEG]), ("dbg_h1", [D, TSEG]),
            ("dbg_xnf", [D, TSEG]),
        ):
            dbg[nm] = nc.dram_tensor(nm, shp, f32, kind="ExternalOutput")

    with tile.TileContext(nc) as tc:
        with (
            tc.tile_pool(name="const", bufs=1) as cpool,
            tc.tile_pool(name="hm", bufs=1) as hm,
            tc.tile_pool(name="scr", bufs=2) as scr,
            tc.tile_pool(name="one", bufs=1) as one,
            tc.tile_pool(name="ps", bufs=1, space="PSUM") as ps,
            tc.tile_pool(name="ps4", bufs=4, space="PSUM") as ps4,
            tc.tile_pool(name="dram", bufs=1, space="DRAM") as dram,
        ):
            # ---------------- constants ----------------
            ident = cpool.tile([128, 128], f32, name="ident")
            make_identity(nc, ident[:, :])
            U_sb = cpool.tile([CH, CH], f32, name="U_sb")
            nc.sync.dma_start(out=U_sb[:, :], in_=d_U[:, :])
            ones4_sb = cpool.tile([NCH, 128], f32, name="ones4_sb")
            nc.sync.dma_start(out=ones4_sb[:, :], in_=d_ones4[:, :])
            ones4f_sb = cpool.tile([NCH, TSEG], f32, name="ones4f_sb")
            nc.sync.dma_start(out=ones4f_sb[:, :], in_=d_ones4f[:, :])
            onesD_sb = cpool.tile([128, 128], f32r, name="onesD_sb")
            nc.sync.dma_start(out=onesD_sb[:, :], in_=d_onesD[:, :])
            eps_sb = cpool.tile([128, 1], f32, name="eps_sb")
            nc.vector.memset(eps_sb[:, :], EPS)
            lnc_sb = []
            for dd in range(ND):
                t = cpool.tile([128, NLC], f32, tag=f"lnc{dd}", name=f"lnc{dd}")
                nc.sync.dma_start(
                    out=t[:, :], in_=d_lnc[dd * 128:(dd + 1) * 128, :])
                lnc_sb.append(t)

            def lncol(key, ll, dd):
                base = {"n1w": 0, "n1b": L, "n2w": 2 * L, "n2b": 3 * L,
                        "Dpw": 4 * L, "Dpb": 5 * L + 2}
                if key == "noww":
                    c = 5 * L
                elif key == "nob":
                    c = 5 * L + 1
                else:
                    c = base[key] + ll
                return lnc_sb[dd][:, c:c + 1]

            # ---------------- h master (feature-major, f32r) --------------
            h = [hm.tile([128, TSEG], f32r, tag=f"h{dd}", name=f"h{dd}")
                 for dd in range(ND)]
            act_ctx = tc.tile_pool(name="act", bufs=1)
            act = act_ctx.__enter__()

            # ---------------- embedding ----------------
            for tt in range(NCH):
                idx_sb = scr.tile([128, 1], i32, tag="idx", name="idx")
                nc.sync.dma_start(
                    out=idx_sb[:, :], in_=d_x[tt * 128:(tt + 1) * 128, :])
                e_t = one.tile([128, D], f32, tag="e_t", name="e_t")
                nc.gpsimd.indirect_dma_start(
                    out=e_t[:, :], out_offset=None, in_=d_emb[:, :],
                    in_offset=bass.IndirectOffsetOnAxis(ap=idx_sb[:, :1], axis=0),
                )
                p_t = one.tile([128, D], f32, tag="p_t", name="p_t")
                nc.sync.dma_start(
                    out=p_t[:, :], in_=d_pos[tt * 128:(tt + 1) * 128, :])
                htm = one.tile([128, D], f32, tag="htm", name="htm")
                nc.vector.tensor_tensor(
                    out=htm[:, :], in0=e_t[:, :], in1=p_t[:, :], op=OP.add)
                for dd in range(ND):
                    trp = ps.tile([128, 128], f32,
                                  tag=("sm" if dd % 2 == 0 else "gps"), name="trp")
                    nc.tensor.transpose(
                        trp[:, :], htm[:, dd * 128:(dd + 1) * 128], ident[:, :])
                    nc.vector.tensor_copy(
                        out=h[dd][:, tt * 128:(tt + 1) * 128], in_=trp[:, :])

            def dump_fm(key, tiles):
                for dd in range(len(tiles)):
                    o = one.tile([128, TSEG], f32, tag="dbgcp", name="dbgcp")
                    nc.vector.tensor_copy(out=o[:, :], in_=tiles[dd][:, :])
                    nc.sync.dma_start(
                        out=dbg[key][dd * 128:(dd + 1) * 128, :], in_=o[:, :])

            if debug:
                dump_fm("dbg_h0", h)

            # ---------------- LN helper ----------------
            def ln_stats():
                # mean and E[x^2] accumulate in parallel (separate banks)
                mu = ps.tile([128, TSEG], f32, tag="sm", name="mu")
                ex2 = ps.tile([128, TSEG], f32, tag="gps", name="ex2")
                for dd in range(ND):
                    nc.tensor.matmul(mu[:, :], onesD_sb[:, :], h[dd][:, :],
                                     start=(dd == 0), stop=(dd == ND - 1))
                for dd in range(ND):
                    h2 = scr.tile([128, TSEG], f32r, tag="h2", name="h2")
                    nc.vector.tensor_tensor(
                        out=h2[:, :], in0=h[dd][:, :], in1=h[dd][:, :], op=OP.mult)
                    nc.tensor.matmul(ex2[:, :], onesD_sb[:, :], h2[:, :],
                                     start=(dd == 0), stop=(dd == ND - 1))
                mu_sb = one.tile([128, TSEG], f32, tag="mu_sb", name="mu_sb")
                nc.vector.tensor_copy(out=mu_sb[:, :], in_=mu[:, :])
                var = one.tile([128, TSEG], f32, tag="var_sb", name="var_sb")
                nc.vector.tensor_tensor(
                    out=var[:, :], in0=mu_sb[:, :], in1=mu[:, :], op=OP.mult)
                nc.vector.tensor_tensor(
                    out=var[:, :], in0=ex2[:, :], in1=var[:, :], op=OP.subtract)
                sd = one.tile([128, TSEG], f32, tag="sd", name="sd")
                nc.scalar.activation(out=sd[:, :], in_=var[:, :], func=AF.Sqrt,
                                     bias=eps_sb[:, :], scale=1.0)
                rstd = one.tile([128, TSEG], f32, tag="rstd", name="rstd")
                nc.vector.reciprocal(out=rstd[:, :], in_=sd[:, :])
                return mu_sb, sd, rstd

            def layer_norm(w_key, b_key, ll=None, out_tag="xn", out_dtype=None):
                mu_sb, sd, rstd = ln_stats()
                hc = []
                for dd in range(ND):
                    c = act.tile([128, TSEG], f32r, tag=f"hc{dd}", name=f"hc{dd}")
                    eng = nc.vector if dd < 2 else nc.gpsimd
                    eng.tensor_tensor(
                        out=c[:, :], in0=h[dd][:, :], in1=mu_sb[:, :],
                        op=OP.subtract)
                    hc.append(c)
                xn = []
                for dd in range(ND):
                    eng = nc.vector if dd < 2 else nc.gpsimd
                    t1 = scr.tile([128, TSEG], f32, tag="lnt1", name="lnt1")
                    eng.tensor_tensor(
                        out=t1[:, :], in0=hc[dd][:, :], in1=rstd[:, :], op=OP.mult)
                    xo = act.tile([128, TSEG], out_dtype or f32r,
                                  tag=f"{out_tag}{dd}", name=f"{out_tag}{dd}")
                    eng.tensor_scalar(
                        out=xo[:, :], in0=t1[:, :],
                        scalar1=lncol(w_key, ll, dd),
                        scalar2=lncol(b_key, ll, dd),
                        op0=OP.mult, op1=OP.add)
                    xn.append(xo)
                return xn

            # ---------------- layers ----------------
            with (
                tc.tile_pool(name="tabs", bufs=1) as tabs,
                tc.tile_pool(name="wbig", bufs=1) as wbig,
            ):
                for ll in range(L):
                    laminv_sb = tabs.tile([CH, S], f32, tag="laminv", name="laminv")
                    lamp_sb = tabs.tile([CH, S], f32, tag="lamp", name="lamp")
                    chc_sb = tabs.tile([CH, S], f32, tag="chc", name="chc")
                    lamw_sb = tabs.tile([1, NCH, S], f32, tag="lamw", name="lamw")
                    chc2T_sb = [tabs.tile([128, TSEG], f32, tag=f"chc2T{ss}",
                                          name=f"chc2T{ss}") for ss in range(NS)]
                    lamc_sb = tabs.tile([NCH, NCH, S], f32, tag="lamc", name="lamc")
                    segc_sb = tabs.tile([NCH, S], f32, tag="segc", name="segc")
                    nc.sync.dma_start(out=laminv_sb[:, :], in_=d_laminv[ll])
                    nc.sync.dma_start(out=lamp_sb[:, :], in_=d_lamp[ll])
                    nc.sync.dma_start(out=chc_sb[:, :], in_=d_chc[ll])
                    nc.sync.dma_start(out=lamw_sb[0:1, :, :], in_=d_lamw[ll])
                    for ss in range(NS):
                        nc.sync.dma_start(
                            out=chc2T_sb[ss][:, :],
                            in_=d_chc2T[ll, ss * 128:(ss + 1) * 128, :])
                    nc.sync.dma_start(out=lamc_sb[:, :, :], in_=d_lamc[ll])
                    nc.sync.dma_start(out=segc_sb[:, :], in_=d_segc[ll])
                    BwT_sb = []
                    for dd in range(ND):
                        t = tabs.tile([128, S], f32r, tag=f"BwT{dd}",
                                      name=f"BwT{dd}")
                        nc.sync.dma_start(
                            out=t[:, :], in_=d_BwT[ll, dd * 128:(dd + 1) * 128, :])
                        BwT_sb.append(t)
                    CwT_sb = []
                    for ss in range(NS):
                        t = tabs.tile([128, D], f32r, tag=f"CwT{ss}",
                                      name=f"CwT{ss}")
                        nc.sync.dma_start(
                            out=t[:, :], in_=d_CwT[ll, ss * 128:(ss + 1) * 128, :])
                        CwT_sb.append(t)
                    w1sb, w2sb = [], []
                    for dd in range(ND):
                        t = wbig.tile([128, DFF], f32r, tag=f"w1sb{dd}",
                                      name=f"w1sb{dd}")
                        nc.sync.dma_start(
                            out=t[:, :], in_=d_w1T[ll, dd * 128:(dd + 1) * 128, :])
                        w1sb.append(t)
                        t = wbig.tile([128, DFF], f32r, tag=f"w2sb{dd}",
                                      name=f"w2sb{dd}")
                        nc.sync.dma_start(
                            out=t[:, :], in_=d_w2T[ll, dd * 128:(dd + 1) * 128, :])
                        w2sb.append(t)

                    # per-layer folded Bw weights (off critical path)
                    BwTw = []
                    for dd in range(ND):
                        t = tabs.tile([128, S], f32r, tag=f"BwTw{dd}",
                                      name=f"BwTw{dd}")
                        nc.gpsimd.tensor_scalar_mul(
                            out=t[:, :], in0=BwT_sb[dd][:, :],
                            scalar1=lncol("n1w", ll, dd))
                        BwTw.append(t)

                    # ---- LN1 stats; P-matmuls start per-chunk on hc ----
                    mu_sb, sd, rstd = ln_stats()
                    # Bb[s] = sum_d n1b[d] BwT[d,s]; applied inside the Bu
                    # matmul as an sd-weighted rank-1 row (so the later *rstd
                    # leaves it unscaled).
                    n1br = []
                    for dd in range(ND):
                        t = scr.tile([128, 1], f32r, tag="n1br", name="n1br")
                        nc.vector.tensor_copy(out=t[:, :],
                                              in_=lncol("n1b", ll, dd))
                        n1br.append(t)
                    colb = ps.tile([CH, S], f32, tag="gps", name="colb")
                    for dd in range(ND):
                        nc.tensor.matmul(colb[0:1, :], n1br[dd][:, :],
                                         BwT_sb[dd][:, :],
                                         start=(dd == 0), stop=(dd == ND - 1))
                    Bb_row = one.tile([1, S], f32, tag="Bb_row", name="Bb_row")
                    nc.vector.tensor_copy(out=Bb_row[:, :], in_=colb[0:1, :])
                    bb_d = dram.tile([1, S], f32, tag="bb_d", name="bb_d")
                    nc.sync.dma_start(out=bb_d[:, :], in_=Bb_row[:, :])
                    Bb_bc = one.tile([CH, S], f32, tag="Bb_bc", name="Bb_bc")
                    nc.gpsimd.dma_start(
                        out=Bb_bc[:, :],
                        in_=bass.AP(tensor=bb_d.tensor, offset=bb_d.offset,
                                    ap=[[0, CH]] + bb_d.ap[1:]))
                    Bblam = one.tile([CH, S], f32, tag="Bblam", name="Bblam")
                    nc.vector.tensor_tensor(
                        out=Bblam[:, :], in0=Bb_bc[:, :], in1=laminv_sb[:, :],
                        op=OP.mult)

                    # sd as per-token columns (PE transpose of bcast rows)
                    sd_col = []
                    for c in range(NCH):
                        trp = ps.tile([128, 128], f32,
                                      tag=("sm" if c % 2 == 0 else "pa"),
                                      name="trpr")
                        nc.tensor.transpose(
                            trp[:, :], sd[:, c * 128:(c + 1) * 128],
                            ident[:, :])
                        col = scr.tile([128, 1], f32, tag="sdc", name="sdc",
                                       bufs=4)
                        nc.vector.reciprocal(out=col[:, :], in_=trp[:, 0:1])
                        sd_col.append(col)

                    # hc, produced chunk-by-chunk so P-matmuls start early
                    hc = [act.tile([128, TSEG], f32r, tag=f"hc{dd}",
                                   name=f"hc{dd}") for dd in range(ND)]
                    for c in range(NCH):
                        for dd in range(ND):
                            eng = nc.vector if dd % 2 == 0 else nc.gpsimd
                            eng.tensor_tensor(
                                out=hc[dd][:, c * 128:(c + 1) * 128],
                                in0=h[dd][:, c * 128:(c + 1) * 128],
                                in1=mu_sb[:, c * 128:(c + 1) * 128],
                                op=OP.subtract)

                    # ---- Bu chunks: v = ((hc@BwTw + sd x Bb) * laminv) * rstd
                    v_all = act.tile([CH, NCH, S], f32, tag="v_all", name="v_all")
                    for c in range(NCH):
                        bu_ps = ps.tile([128, TSEG], f32,
                                        tag=("pa" if c % 2 == 0 else "pb"),
                                        name="bu_ps")
                        for dd in range(ND):
                            nc.tensor.matmul(
                                bu_ps[:, :S],
                                hc[dd][:, c * 128:(c + 1) * 128],
                                BwTw[dd][:, :],
                                start=(dd == 0), stop=(dd == ND - 1))
                        vt = scr.tile([CH, S], f32, tag="vt", name="vt")
                        nc.vector.tensor_tensor(
                            out=vt[:, :], in0=bu_ps[:, :S], in1=laminv_sb[:, :],
                            op=OP.mult)
                        nc.vector.tensor_scalar_mul(
                            out=vt[:, :], in0=vt[:, :],
                            scalar1=sd_col[c][:, :])
                        nc.vector.tensor_tensor(
                            out=v_all[:, c, :], in0=vt[:, :], in1=Bblam[:, :],
                            op=OP.add)
                        # G partial: column-sum of v chunk
                        gcol = ps.tile([CH, S], f32, tag="gps", name="gcol")
                        nc.tensor.matmul(gcol[0:1, :], U_sb[:, 127:128],
                                         v_all[:, c, :], start=True, stop=True)
                        gt = scr.tile([1, S], f32, tag="gt", name="gt")
                        nc.vector.tensor_tensor(
                            out=gt[:, :], in0=gcol[0:1, :],
                            in1=lamw_sb[0:1, c, :], op=OP.mult)
                        if c == 0:
                            G_sb = one.tile([1, S], f32, tag="G_sb", name="G_sb")
                            nc.vector.tensor_copy(out=G_sb[:, :], in_=gt[:, :])
                        else:
                            nc.vector.tensor_tensor(
                                out=G_sb[:, :], in0=G_sb[:, :], in1=gt[:, :],
                                op=OP.add)

                    # launch AllGather of local-final state ASAP
                    g_in = dram.tile([1, S], f32, tag="g_in", name="g_in")
                    g_out = dram.tile([NCH, S], f32, tag="g_out", name="g_out")
                    nc.sync.dma_start(out=g_in[:, :], in_=G_sb[:, :])
                    nc.gpsimd.collective_compute(
                        "AllGather", mybir.AluOpType.bypass,
                        replica_groups=[[0, 1, 2, 3], [4, 5, 6, 7]],
                        ins=[g_in.opt()], outs=[g_out.opt()],
                    )

                    # ---- intra-chunk cumsums (overlap the collective) ----
                    intra = act.tile([CH, NCH, S], f32, tag="intra", name="intra")
                    for c in range(NCH):
                        cum = ps.tile([CH, S], f32, tag="sm", name="cum")
                        nc.tensor.matmul(cum[:, :], U_sb[:, :], v_all[:, c, :],
                                         start=True, stop=True)
                        nc.vector.tensor_tensor(
                            out=intra[:, c, :], in0=cum[:, :], in1=lamp_sb[:, :],
                            op=OP.mult)

                    # ---- chunk-carry fixup (local) ----
                    S4 = one.tile([NCH, S], f32, tag="S4", name="S4")
                    nc.sync.dma_start(out=S4[:, :], in_=intra[CH - 1:CH, :, :])
                    for c in range(1, NCH):
                        rows = scr.tile([NCH, S], f32, tag="rows", name="rows")
                        nc.vector.tensor_tensor(
                            out=rows[:, :], in0=S4[:, :], in1=lamc_sb[:, c, :],
                            op=OP.mult)
                        pfix = ps.tile([CH, S], f32, tag="sm", name="pfix")
                        nc.tensor.matmul(pfix[:, :], ones4_sb[:, :], rows[:, :],
                                         start=True, stop=True)
                        tmp = scr.tile([CH, S], f32, tag="fixt", name="fixt")
                        nc.vector.tensor_tensor(
                            out=tmp[:, :], in0=pfix[:, :], in1=chc_sb[:, :],
                            op=OP.mult)
                        nc.vector.tensor_tensor(
                            out=intra[:, c, :], in0=intra[:, c, :], in1=tmp[:, :],
                            op=OP.add)

                    if debug and ll == 0:
                        o3 = one.tile([128, NCH * S], f32, tag="dbgcp3",
                                      name="dbgcp3")
                        nc.vector.tensor_copy(
                            out=o3[:, :],
                            in_=intra[:, :, :].rearrange("p a b -> p (a b)"))
                        nc.sync.dma_start(out=dbg["dbg_loc"][:, :], in_=o3[:, :])

                    # ---- transpose hscan -> [s, t] (still during collective) --
                    hsT = [act.tile([128, TSEG], f32r, tag=f"hsT{ss}",
                                    name=f"hsT{ss}") for ss in range(NS)]
                    for c in range(NCH):
                        for ss in range(NS):
                            trp = ps.tile([128, 128], f32,
                                          tag=("sm" if (c * NS + ss) % 2 == 0
                                               else "gps"), name="trp2")
                            nc.tensor.transpose(
                                trp[:, :], intra[:, c, ss * 128:(ss + 1) * 128],
                                ident[:, :])
                            nc.vector.tensor_copy(
                                out=hsT[ss][:, c * 128:(c + 1) * 128],
                                in_=trp[:, :])

                    if debug and ll == 0:
                        dump_fm("dbg_hst", hsT)

                    # ---- cross-core carry folded into the C projection ----
                    # dh[d,t] = sum_s Cw[d,s]*E[s]*chc2T[s,t]; E from Gall.
                    Gall = one.tile([NCH, S], f32, tag="Gall", name="Gall")
                    nc.sync.dma_start(out=Gall[:, :], in_=g_out[:, :])
                    rowsE = scr.tile([NCH, S], f32, tag="rowsE", name="rowsE")
                    nc.vector.tensor_tensor(
                        out=rowsE[:, :], in0=Gall[:, :], in1=segc_sb[:, :],
                        op=OP.mult)
                    for ss in range(NS):
                        Ebc = ps.tile([128, TSEG], f32,
                                      tag=("pa" if ss == 0 else "pb"), name="Ebc")
                        nc.tensor.matmul(
                            Ebc[:, :], rowsE[:, ss * 128:(ss + 1) * 128],
                            ones4f_sb[:, :], start=True, stop=True)
                        ctmp = scr.tile([128, TSEG], f32, tag="ctmp", name="ctmp")
                        nc.vector.tensor_tensor(
                            out=ctmp[:, :], in0=Ebc[:, :], in1=chc2T_sb[ss][:, :],
                            op=OP.mult)
                        nc.vector.tensor_tensor(
                            out=hsT[ss][:, :], in0=hsT[ss][:, :], in1=ctmp[:, :],
                            op=OP.add)
                    for dd in range(ND):
                        cp_ps = ps4.tile([128, TSEG], f32, tag="acc",
                                         name="cp_ps")
                        for ss in range(NS):
                            nc.tensor.matmul(
                                cp_ps[:, :],
                                CwT_sb[ss][:, dd * 128:(dd + 1) * 128],
                                hsT[ss][:, :],
                                start=(ss == 0), stop=(ss == NS - 1))
                        eng = nc.vector if dd < 2 else nc.gpsimd
                        t2 = scr.tile([128, TSEG], f32, tag="t2du", name="t2du")
                        eng.tensor_tensor(
                            out=t2[:, :], in0=hc[dd][:, :], in1=rstd[:, :],
                            op=OP.mult)
                        du = scr.tile([128, TSEG], f32, tag="du", name="du")
                        eng.tensor_scalar(
                            out=du[:, :], in0=t2[:, :],
                            scalar1=lncol("Dpw", ll, dd),
                            scalar2=lncol("Dpb", ll, dd),
                            op0=OP.mult, op1=OP.add)
                        nc.vector.tensor_tensor(
                            out=h[dd][:, :], in0=h[dd][:, :], in1=cp_ps[:, :],
                            op=OP.add)
                        nc.vector.tensor_tensor(
                            out=h[dd][:, :], in0=h[dd][:, :], in1=du[:, :],
                            op=OP.add)

                    if debug and ll == 0:
                        dump_fm("dbg_h1", h)

                    # ---- LN2 ----
                    xn2 = layer_norm("n2w", "n2b", ll, out_tag="xm")

                    # ---- SwiGLU ----
                    sw_ps = [ps4.tile([128, TSEG], f32, tag="acc", name="sw_ps")
                             for _ in range(ND)]
                    f0 = 0
                    for fi, pf in enumerate(FTS):
                        a_ps = ps.tile([128, TSEG], f32, tag="pa", name="a_ps")
                        b_ps = ps.tile([128, TSEG], f32, tag="pb", name="b_ps")
                        for dd in range(ND):
                            nc.tensor.matmul(
                                a_ps[:pf, :], w1sb[dd][:, f0:f0 + pf],
                                xn2[dd][:, :],
                                start=(dd == 0), stop=(dd == ND - 1))
                        for dd in range(ND):
                            nc.tensor.matmul(
                                b_ps[:pf, :], w2sb[dd][:, f0:f0 + pf],
                                xn2[dd][:, :],
                                start=(dd == 0), stop=(dd == ND - 1))
                        sa = scr.tile([128, TSEG], f32r, tag="sa", name="sa")
                        nc.scalar.activation(out=sa[:pf, :], in_=a_ps[:pf, :],
                                             func=AF.Silu)
                        g = scr.tile([128, TSEG], f32r, tag="g", name="g")
                        nc.vector.tensor_tensor(
                            out=g[:pf, :], in0=sa[:pf, :], in1=b_ps[:pf, :],
                            op=OP.mult)
                        w3t = scr.tile([128, D], f32r, tag="w3t", name="w3t")
                        nc.sync.dma_start(
                            out=w3t[:pf, :], in_=d_w3T[ll, f0:f0 + pf, :])
                        for dd in range(ND):
                            nc.tensor.matmul(
                                sw_ps[dd][:, :],
                                w3t[:pf, dd * 128:(dd + 1) * 128],
                                g[:pf, :],
                                start=(fi == 0), stop=(fi == NFT - 1))
                        f0 += pf
                    for dd in range(ND):
                        nc.vector.tensor_tensor(
                            out=h[dd][:, :], in0=h[dd][:, :], in1=sw_ps[dd][:, :],
                            op=OP.add)

            # ---------------- final LN (bf16 out for the bf16 head) ------
            xnf = layer_norm("noww", "nob", None, out_tag="xn",
                             out_dtype=bf16)
            if debug:
                dump_fm("dbg_xnf", xnf)

            # ---------------- head (token-sharded, full vocab) -----------
            # Per vocab block vb: one batched DMA loads [128d, 4dd x nv] of
            # bf16 weights; nv/128 PSUM groups of [128v, 512t]; bias+copy on
            # the ACT engine; one batched DMA writes [nv, 512] bf16 logits.
            with tc.tile_pool(name="hd", bufs=1) as hd, \
                 tc.tile_pool(name="hw2", bufs=3) as hw2, \
                 tc.tile_pool(name="ho", bufs=3) as ho:
                hbc_sb = hd.tile([128, NVT], f32, tag="hbc", name="hbc")
                nc.sync.dma_start(out=hbc_sb[:, :], in_=d_hbc[:, :])
                for vb in range(NVB):
                    v0 = vb * 512
                    nv = min(512, V - v0)
                    nsub = nv // 128
                    wt = hw2.tile([128, 4 * 512], bf16, tag="wt", name="wt")
                    hw_ap = d_hWp[:, :]
                    nc.sync.dma_start(
                        out=wt[:, :4 * nv],
                        in_=bass.AP(tensor=hw_ap.tensor,
                                    offset=hw_ap.offset + v0,
                                    ap=[[4 * V, 128], [V, 4], [1, nv]]))
                    ot = ho.tile([128, 4 * TSEG], bf16, tag="ot", name="ot")
                    for sub in range(nsub):
                        hp_ps = ps4.tile([128, TSEG], f32, tag="acc",
                                         name="hp_ps")
                        for dd in range(ND):
                            nc.tensor.matmul(
                                hp_ps[:, :],
                                wt[:, dd * nv + sub * 128:
                                   dd * nv + sub * 128 + 128],
                                xnf[dd][:, :],
                                start=(dd == 0), stop=(dd == ND - 1))
                        nc.scalar.activation(
                            out=ot[:, sub * TSEG:(sub + 1) * TSEG],
                            in_=hp_ps[:, :], func=AF.Identity,
                            bias=hbc_sb[:, vb * 4 + sub:vb * 4 + sub + 1],
                            scale=1.0)
                    out_ap = d_out[:, :]
                    nc.sync.dma_start(
                        out=bass.AP(tensor=out_ap.tensor,
                                    offset=out_ap.offset + v0 * TSEG,
                                    ap=[[TSEG, 128], [128 * TSEG, nsub],
                                        [1, TSEG]]),
                        in_=ot[:, :nsub * TSEG])

            act_ctx.__exit__(None, None, None)

    nc.compile()
    return nc


def _host_prep(inputs):
    """Build the 8 per-core input maps from full inputs."""
    x = np.asarray(inputs["x"]).astype(np.int32)
    emb = np.asarray(inputs["emb"], np.float32)
    pos = np.asarray(inputs["pos"], np.float32)
    lam = 1.0 / (1.0 + np.exp(-np.asarray(inputs["log_lambda"], np.float64)))
    Bw = np.asarray(inputs["Bw"], np.float32)
    Cw = np.asarray(inputs["Cw"], np.float32)
    w1 = np.asarray(inputs["w1"], np.float32)
    w2 = np.asarray(inputs["w2"], np.float32)
    w3 = np.asarray(inputs["w3"], np.float32)
    headW = np.asarray(inputs["headW"], np.float32)
    headb = np.asarray(inputs["headb"], np.float32)

    BwT = np.ascontiguousarray(Bw.transpose(0, 2, 1))
    CwT = np.ascontiguousarray(Cw.transpose(0, 2, 1))
    w1T = np.ascontiguousarray(w1.transpose(0, 2, 1))
    w2T = np.ascontiguousarray(w2.transpose(0, 2, 1))
    w3T = np.ascontiguousarray(w3.transpose(0, 2, 1))

    import ml_dtypes
    # hWp[p, dd*V + v] = headW[v, dd*128 + p]  (bf16)
    hWp = np.ascontiguousarray(
        headW.T.reshape(ND, 128, V).transpose(1, 0, 2).reshape(128, 4 * V)
    ).astype(ml_dtypes.bfloat16)
    # hbc[p, t] = headb[t*128 + p]
    hbc = np.ascontiguousarray(headb.reshape(NVT, 128).T)

    # packed LN params:
    # [n1w(L), n1b(L), n2w(L), n2b(L), Dp*n1w(L), now, nob, Dp*n1b(L)]
    Dp = np.asarray(inputs["Dp"], np.float32)
    n1w = np.asarray(inputs["n1w"], np.float32)
    n1b = np.asarray(inputs["n1b"], np.float32)
    lncols = np.zeros((D, NLC), np.float32)
    for i, arr in enumerate((n1w, n1b,
                             np.asarray(inputs["n2w"], np.float32),
                             np.asarray(inputs["n2b"], np.float32),
                             Dp * n1w)):
        lncols[:, i * L:(i + 1) * L] = arr.T
    lncols[:, 5 * L] = np.asarray(inputs["now"], np.float32)
    lncols[:, 5 * L + 1] = np.asarray(inputs["nob"], np.float32)
    lncols[:, 5 * L + 2:6 * L + 2] = (Dp * n1b).T

    i_ar = np.arange(CH, dtype=np.float64)[None, :, None]  # [1, CH, 1]
    lamB = lam[:, None, :]                                 # [L, 1, S]
    laminv = (lamB ** (-i_ar)).astype(np.float32)
    lamp = (lamB ** i_ar).astype(np.float32)
    chc = (lamB ** (i_ar + 1)).astype(np.float32)
    lamw = np.zeros((L, NCH, S), np.float32)
    for c in range(NCH):
        lamw[:, c, :] = (lam ** (TSEG - 1 - CH * c)).astype(np.float32)
    t_ar = np.arange(TSEG, dtype=np.float64)[None, None, :]  # [1, 1, T]
    chc2T = (lam[:, :, None] ** (t_ar + 1)).astype(np.float32)  # [L, S, T]
    lamc = np.zeros((L, NCH, NCH, S), np.float32)
    for c in range(1, NCH):
        for cp in range(c):
            lamc[:, cp, c, :] = (lam ** (CH * (c - 1 - cp))).astype(np.float32)
    U = np.triu(np.ones((CH, CH), np.float32))
    ones4 = np.ones((NCH, 128), np.float32)
    ones4f = np.ones((NCH, TSEG), np.float32)
    onesD = np.full((128, 128), 1.0 / D, np.float32)

    in_maps = []
    for k in range(NCORES):
        b, r = divmod(k, NSEG)
        t0 = r * TSEG
        segcoef = np.zeros((L, NCH, S), np.float32)
        for sp in range(r):
            segcoef[:, sp, :] = (lam ** (TSEG * (r - 1 - sp))).astype(np.float32)
        in_maps.append({
            "x_seg": np.ascontiguousarray(x[b, t0:t0 + TSEG, None]),
            "emb": emb,
            "pos_seg": np.ascontiguousarray(pos[t0:t0 + TSEG]),
            "BwT": BwT, "CwT": CwT, "w1T": w1T, "w2T": w2T, "w3T": w3T,
            "hWp": hWp, "hbc": hbc,
            "lncols": lncols,
            "laminv": laminv, "lamp": lamp, "chc": chc, "lamw": lamw,
            "chc2T": chc2T, "lamc": lamc, "segcoef": segcoef,
            "Utri": U, "ones4": ones4, "ones4f": ones4f, "onesD": onesD,
        })
    return in_maps


def kernel(**inputs) -> np.ndarray:
    from concourse.bass_utils import run_bass_kernel_spmd

    if "nc" not in _NC_CACHE:
        _NC_CACHE["nc"] = _build_nc()
    nc = _NC_CACHE["nc"]
    in_maps = _host_prep(inputs)
    res = None
    last_err = None
    for _attempt in range(3):
        try:
            res = run_bass_kernel_spmd(nc, in_maps, core_ids=list(range(NCORES)))
            break
        except Exception as e:  # transient device hiccups: retry
            last_err = e
    if res is None:
        raise last_err
    return assemble_output([res.results[k]["logits"] for k in range(NCORES)])


def assemble_output(parts):
    """parts[k]: [V, TSEG] bf16 logits for core k's tokens -> [B, T, V] f32."""
    out = np.empty((B, T, V), np.float32)
    for k in range(NCORES):
        b, r = divmod(k, NSEG)
        out[b, r * TSEG:(r + 1) * TSEG] = parts[k].astype(np.float32).T
    return out

